# revision 2
# baseline (speedup 1.0000x reference)
"""Trainium2 kernel v2 for the dense transformer block (B=4, T=2048, C=1024,
H=16, MLP 4x, hybrid cond/causal mask), SPMD over 8 cores, collective-free.

Core (b, rho) = (core//2, core%2) handles batch b, query chunks {2*qi+rho}.
The T axis is chunk-permuted per core (pair-swap for rho=0) so ONE program
(query chunk at position 2*qi+1, kv limit 2*qi+2) serves both rho values.
All GEMMs run in fp8e4m3 DoubleRow (weights scaled 2^12 host-side); masks are
added as -240 bias via PE identity-matmuls before the softmax exp.
"""
import sys
sys.path.insert(0, '/opt/trn_rl_repo')
import numpy as np
import ml_dtypes
import concourse.bass as bass
import concourse.mybir as mybir
import concourse.tile as tile
from concourse.vector_clock import ScopedClock
from concourse.bass_utils import run_bass_kernel_spmd

# ---- walrus 1-sync-wait-per-instruction workarounds (from baseline) ----
_installed = False


def _split_multi_waits(ordered_by_block, nc):
    for bb_name, insts in ordered_by_block.items():
        need = any(
            inst.sync_info is not None and len(inst.sync_info.on_wait) > 1
            for inst in insts
        )
        if not need:
            continue
        new_list = []
        for inst in insts:
            si = inst.sync_info
            waits = list(si.on_wait) if si is not None and si.on_wait else []
            if len(waits) > 1:
                for w in waits[:-1]:
                    nop = mybir.InstNoOp(
                        name=nc.get_next_instruction_name(),
                        ins=[],
                        outs=[],
                        bass_nofuse=True,
                    )
                    nop.engine = inst.engine
                    nop.sync_info = mybir.SyncInfo(on_wait=[w], on_update=[])
                    new_list.append(nop)
                ups = list(si.on_update) if si.on_update else []
                inst.sync_info = mybir.SyncInfo(on_wait=[waits[-1]], on_update=ups)
            new_list.append(inst)
        insts[:] = new_list


class _SplitWaitClockWait:
    def __init__(self, tc, ordered, **kw):
        import bass_rust
        self._inner = bass_rust.TileClockWait(tc, ordered, **kw)
        self._tc = tc
        self._ordered = ordered

    def __getattr__(self, a):
        return getattr(self._inner, a)

    def assign_waits(self, bb_name):
        r = self._inner.assign_waits(bb_name)
        _split_multi_waits(self._ordered, self._tc.nc)
        return r


class PatchedTileContext(tile.TileContext):
    """TileContext whose final drain carries at most one sem wait."""

    def _drain_and_barrier(self, tick_clock, wait_clock):
        probe = self.nc.sync.nop(nofuse=True)
        add = wait_clock.add_sem_waits
        add(probe.ins, ScopedClock({None: tick_clock.global_clock}))
        si = probe.ins.sync_info
        waits = list(si.on_wait) if si is not None and si.on_wait else []
        if len(waits) > 1:
            probe.ins.sync_info = mybir.SyncInfo(on_wait=[waits[0]], on_update=[])
            for w in waits[1:]:
                n = self.nc.sync.nop(nofuse=True)
                n.ins.sync_info = mybir.SyncInfo(on_wait=[w], on_update=[])
        self.nc.sync.drain()
        self.nc.all_engine_barrier()
        popped = self.nc._tile_sem_poison_stack.pop()
        assert popped is self._sem_poison
        self.nc.clear_and_free_semaphores(list(self.sems.allocated().values()))
        self.nc.all_engine_barrier()


def _install():
    global _installed
    if not _installed:
        tile.TileClockWait = _SplitWaitClockWait
        _installed = True


_install()

# ---- constants ----

F32 = mybir.dt.float32
BF16 = mybir.dt.bfloat16
FP8 = mybir.dt.float8e4
E4 = ml_dtypes.float8_e4m3
AF = mybir.ActivationFunctionType
ALU = mybir.AluOpType
DRM = mybir.MatmulPerfMode.DoubleRow

C = 1024
T = 2048
H = 16
HD = 64
FF = 4096
COND = 256
EPS = 1e-5
SCALE = 1.0 / np.sqrt(HD)
WS = 2.0 ** 12       # weight scale
WSI = float(2.0 ** -12)
MNEG = -240.0
NQI = 8              # query chunks per core
# cvec column indices in the packed [128, 88] bias tensor
CV = {"bq": 0, "bp": 8, "b2": 16, "g1": 24, "bt1": 32, "g2": 40, "bt2": 48, "b1": 56}


def build_nc(debug=False):
    nc = bass.Bass()
    xT = nc.dram_tensor("xT", [C, T], F32, kind="ExternalInput")
    wq = nc.dram_tensor("wq", [128, 8, 4, 2, 128], FP8, kind="ExternalInput")
    wk = nc.dram_tensor("wk", [128, 8, 4, 2, 128], FP8, kind="ExternalInput")
    wv = nc.dram_tensor("wv", [128, 4, 4, 2, 256], FP8, kind="ExternalInput")
    wp = nc.dram_tensor("wp", [128, 8, 4, 2, 128], FP8, kind="ExternalInput")
    w1 = nc.dram_tensor("w1", [128, 32, 4, 2, 128], FP8, kind="ExternalInput")
    w2 = nc.dram_tensor("w2", [128, 8, 16, 2, 128], FP8, kind="ExternalInput")
    cvec = nc.dram_tensor("cvec", [128, 88], F32, kind="ExternalInput")
    bvb = nc.dram_tensor("bvb", [128, C], BF16, kind="ExternalInput")
    mneg = nc.dram_tensor("mneg", [14 * 128, 128], FP8, kind="ExternalInput")
    i128 = nc.dram_tensor("i128", [128, 128], FP8, kind="ExternalInput")
    outT = nc.dram_tensor("outT", [C, 1024], F32, kind="ExternalOutput")

    stats_d = nc.dram_tensor("stats_d", [2, T + 1024], BF16)   # a=rstd rows, b=-mu*rstd
    rs_d = nc.dram_tensor("rs_d", [16, 1024], BF16)           # recip sums per (qi, half)

    dbg = {}
    if debug:
        dbg["hT"] = nc.dram_tensor("dbg_hT", [C, T], F32, kind="ExternalOutput")
        dbg["v"] = nc.dram_tensor("dbg_v", [T, H * HD], F32, kind="ExternalOutput")
        dbg["yT"] = nc.dram_tensor("dbg_yT", [C, 1024], F32, kind="ExternalOutput")
        dbg["x2T"] = nc.dram_tensor("dbg_x2T", [C, 1024], F32, kind="ExternalOutput")

    with PatchedTileContext(nc) as tc:
        import contextlib
        with contextlib.ExitStack() as _es:
            _e = _es.enter_context
            _e(nc.allow_low_precision(reason="bf16/fp8 intermediates validated vs numpy"))
            big = _e(tc.tile_pool(name="big", bufs=1))
            wbig = _e(tc.tile_pool(name="wbig", bufs=1))
            w1s_p = _e(tc.tile_pool(name="w1s", bufs=2))
            w2s_p = _e(tc.tile_pool(name="w2s", bufs=2))
            st = _e(tc.tile_pool(name="st", bufs=1))
            xin_p = _e(tc.tile_pool(name="xin", bufs=3))
            xb_p = _e(tc.tile_pool(name="xb", bufs=1))
            bc_p = _e(tc.tile_pool(name="bc", bufs=1))
            rows_p = _e(tc.tile_pool(name="rows", bufs=1))
            se_p = _e(tc.tile_pool(name="se", bufs=2))
            ktmp_p = _e(tc.tile_pool(name="ktmp", bufs=2))
            sml_p = _e(tc.tile_pool(name="sml", bufs=2))
            out_p = _e(tc.tile_pool(name="outp", bufs=2))
            pss = _e(tc.tile_pool(name="pss", bufs=1, space="PSUM"))
            psy = _e(tc.tile_pool(name="psy", bufs=1, space="PSUM"))
            ps1 = _e(tc.tile_pool(name="ps1", bufs=2, space="PSUM"))
            # ---------------- resident tensors ----------------
            hT = big.tile([128, 8, T], FP8, tag="A")
            v_ext = big.tile([128, 16, 16, 80], FP8, tag="V")
            kst = [big.tile([128, 2, T], FP8, tag=f"K{t4}", name=f"kst{t4}") for t4 in range(4)]
            qst = [big.tile([128, 2, NQI, 128], FP8, tag=f"Q{t4}", name=f"qst{t4}") for t4 in range(4)]
            yT = big.tile([128, 8, 1024], FP8, tag="Y")
            x2 = big.tile([128, 8, 1024], BF16, tag="X2")
            h2T = big.tile([128, 8, 1024], FP8, tag="H2")
            wq_sb = wbig.tile([128, 8, 4, 2, 128], FP8, tag="WQ")
            wk_sb = wbig.tile([128, 8, 4, 2, 128], FP8, tag="WK")
            wv_sb = wbig.tile([128, 4, 4, 2, 256], FP8, tag="WV")
            wp_sb = wbig.tile([128, 8, 4, 2, 128], FP8, tag="WP")
            cv = st.tile([128, 88], F32)
            bvb_sb = st.tile([128, C], BF16)
            mneg_sb = st.tile([128, 14, 128], FP8)
            i128_sb = st.tile([128, 128], FP8)
            ones1 = st.tile([128, 1], BF16)
            eps1 = st.tile([1, 1], F32)

            nc.vector.memset(ones1, 1.0)
            nc.vector.memset(eps1, EPS)
            nc.sync.dma_start(out=wq_sb, in_=wq[:, :, :, :, :])
            nc.sync.dma_start(out=wk_sb, in_=wk[:, :, :, :, :])
            nc.sync.dma_start(out=wv_sb, in_=wv[:, :, :, :, :])
            nc.sync.dma_start(out=wp_sb, in_=wp[:, :, :, :, :])
            nc.sync.dma_start(out=cv, in_=cvec[:, :])
            nc.sync.dma_start(out=bvb_sb, in_=bvb[:, :])
            nc.sync.dma_start(out=mneg_sb, in_=mneg.rearrange("(s p) k -> p s k", p=128))
            nc.sync.dma_start(out=i128_sb, in_=i128[:, :])

            def cvap(name, i):
                return cv[:, CV[name] + i:CV[name] + i + 1]

            # ---------------- layernorm (x source: dram f32 or sbuf bf16) ------
            def ln(src_dram, src_sb, col0, ncols, stat_off, out_tile, gname, btname):
                """src [C, *]; normalizes cols [col0, col0+ncols) writing fp8
                (affine g/bt) into out_tile[:, cc, same cols]. Stats broadcast
                via stats_d rows at stat_off+col."""
                nt = ncols // 512
                for t in range(nt):
                    cs = slice(col0 + 512 * t, col0 + 512 * (t + 1))
                    mu_ps = ps1.tile([1, 512], F32, tag="pmid")
                    sq_ps = ps1.tile([1, 512], F32, tag="pmid")
                    xbt = xb_p.tile([128, 8, 512], BF16, tag="xb")
                    for cc in range(8):
                        if src_dram is not None:
                            xt = xin_p.tile([128, 512], F32, tag="x")
                            nc.sync.dma_start(
                                out=xt, in_=src_dram[128 * cc:128 * (cc + 1), cs])
                            nc.vector.tensor_copy(out=xbt[:, cc, :], in_=xt)
                        else:
                            nc.vector.tensor_copy(
                                out=xbt[:, cc, :], in_=src_sb[:, cc, cs])
                        sq = xin_p.tile([128, 512], BF16, tag="sq")
                        nc.vector.tensor_mul(sq, xbt[:, cc, :], xbt[:, cc, :])
                        nc.tensor.matmul(mu_ps, ones1, xbt[:, cc, :],
                                         start=(cc == 0), stop=(cc == 7))
                        nc.tensor.matmul(sq_ps, ones1, sq,
                                         start=(cc == 0), stop=(cc == 7))
                    mu = rows_p.tile([1, 512], F32, tag="mu")
                    ex2 = rows_p.tile([1, 512], F32, tag="t2")
                    nc.vector.tensor_scalar_mul(mu, mu_ps, 1.0 / C)
                    nc.vector.tensor_scalar_mul(ex2, sq_ps, 1.0 / C)
                    var = rows_p.tile([1, 512], F32, tag="t3")
                    nc.vector.scalar_tensor_tensor(
                        out=var, in0=mu, scalar=-1.0, in1=mu, op0=ALU.mult, op1=ALU.mult)
                    nc.vector.tensor_add(var, var, ex2)
                    std = rows_p.tile([1, 512], F32, tag="t2")
                    nc.scalar.activation(out=std, in_=var, func=AF.Sqrt, bias=eps1)
                    arow = rows_p.tile([1, 512], BF16, tag="ab16")
                    nc.vector.reciprocal(out=arow, in_=std)
                    brow = rows_p.tile([1, 512], BF16, tag="ab16b")
                    nc.vector.scalar_tensor_tensor(
                        out=brow, in0=mu, scalar=-1.0, in1=arow,
                        op0=ALU.mult, op1=ALU.mult)
                    so = stat_off + col0 + 512 * t
                    nc.sync.dma_start(out=stats_d[0, so:so + 512][None, :], in_=arow)
                    nc.sync.dma_start(out=stats_d[1, so:so + 512][None, :], in_=brow)
                    a_b = bc_p.tile([128, 512], BF16, tag="a_b")
                    b_b = bc_p.tile([128, 512], BF16, tag="b_b")
                    nc.sync.dma_start(out=a_b, in_=bass.AP(
                        tensor=stats_d[0][None, :].tensor,
                        offset=so, ap=[[0, 128], [1, 512]]))
                    nc.sync.dma_start(out=b_b, in_=bass.AP(
                        tensor=stats_d[1][None, :].tensor,
                        offset=(T + 1024) + so, ap=[[0, 128], [1, 512]]))
                    for cc in range(8):
                        t1 = xin_p.tile([128, 512], BF16, tag="t1")
                        nc.vector.tensor_mul(t1, xbt[:, cc, :], a_b)
                        u = xin_p.tile([128, 512], BF16, tag="u")
                        nc.vector.tensor_add(u, t1, b_b)
                        nc.gpsimd.tensor_scalar(
                            out_tile[:, cc, cs], u, cvap(gname, cc), cvap(btname, cc),
                            op0=ALU.mult, op1=ALU.add)

            ln(xT, None, 0, T, 0, hT, "g1", "bt1")

            if debug:
                for cc in range(8):
                    for t in range(4):
                        dt_ = out_p.tile([128, 512], F32, tag="dbg")
                        nc.vector.tensor_copy(out=dt_, in_=hT[:, cc, 512 * t:512 * (t + 1)])
                        nc.sync.dma_start(
                            out=dbg["hT"][128 * cc:128 * (cc + 1), 512 * t:512 * (t + 1)],
                            in_=dt_)

            # ---------------- V (t-partition layout, +bv, ones col) ------------
            for fh in range(4):
                for m in range(16):
                    vps = ps1.tile([128, 256], F32, tag="pmid")
                    for g in range(4):
                        nc.tensor.matmul(
                            vps, hT[:, 2 * g:2 * g + 2, 128 * m:128 * (m + 1)],
                            wv_sb[:, fh, g, :, :],
                            start=(g == 0), stop=(g == 3), perf_mode=DRM,
                            skip_group_check=True)
                    nc.vector.scalar_tensor_tensor(
                        out=v_ext[:, m, 4 * fh:4 * fh + 4, 0:HD],
                        in0=vps.rearrange("p (h d) -> p h d", h=4), scalar=WSI,
                        in1=bvb_sb[:, 256 * fh:256 * (fh + 1)].rearrange(
                            "p (h d) -> p h d", h=4),
                        op0=ALU.mult, op1=ALU.add)
            for m in range(16):
                nc.vector.memset(v_ext[:, m, :, HD:HD + 1], 1.0)

            if debug:
                for m in range(16):
                    dt_ = out_p.tile([128, H * HD], F32, tag="dbgv")
                    nc.vector.tensor_copy(
                        out=dt_.rearrange("p (h d) -> p h d", h=H),
                        in_=v_ext[:, m, :, 0:HD])
                    nc.sync.dma_start(out=dbg["v"][128 * m:128 * (m + 1), :], in_=dt_)

            # ---------------- K (then remap to kst) ----------------
            for p in range(8):
                ktmp = ktmp_p.tile([128, 2048], FP8, tag="kt")
                for t in range(4):
                    kps = ps1.tile([128, 512], F32, tag="pmid")
                    for half in range(2):
                        cs = slice(512 * t + 256 * half, 512 * t + 256 * (half + 1))
                        for g in range(4):
                            nc.tensor.matmul(
                                kps[:, 256 * half:256 * (half + 1)],
                                wk_sb[:, p, g, :, :], hT[:, 2 * g:2 * g + 2, cs],
                                start=(g == 0), stop=(g == 3), perf_mode=DRM,
                                skip_group_check=True)
                    nc.vector.tensor_scalar_mul(
                        ktmp[:, 512 * t:512 * (t + 1)], kps, WSI)
                for odd in range(2):
                    h = 2 * p + odd
                    t4, h4 = h // 4, h % 4
                    for i in range(2):
                        nc.sync.dma_start(
                            out=kst[t4][32 * h4:32 * h4 + 32, i, :],
                            in_=ktmp[64 * odd + 32 * i:64 * odd + 32 * i + 32, :])

            # ---------------- Q (then remap to qst) ----------------
            for p in range(8):
                qtmp = ktmp_p.tile([128, 8, 128], FP8, tag="qt")
                for qh in range(2):
                    qps = ps1.tile([128, 4, 128], F32, tag="pmid")
                    for qi4 in range(4):
                        qi = 4 * qh + qi4
                        qc0 = 128 * (2 * qi + 1)
                        for g in range(4):
                            nc.tensor.matmul(
                                qps[:, qi4, :], wq_sb[:, p, g, :, :],
                                hT[:, 2 * g:2 * g + 2, qc0:qc0 + 128],
                                start=(g == 0), stop=(g == 3), perf_mode=DRM,
                                skip_group_check=True)
                    nc.vector.tensor_scalar(
                        qtmp[:, 4 * qh:4 * qh + 4, :].rearrange("p a b -> p (a b)"),
                        qps.rearrange("p a b -> p (a b)"),
                        WSI, cvap("bq", p), op0=ALU.mult, op1=ALU.add)
                for odd in range(2):
                    h = 2 * p + odd
                    t4, h4 = h // 4, h % 4
                    for i in range(2):
                        nc.sync.dma_start(
                            out=qst[t4][32 * h4:32 * h4 + 32, i, :, :],
                            in_=qtmp[64 * odd + 32 * i:64 * odd + 32 * i + 32, :, :])

            # ---------------- attention + proj + LN2 + MLP, pipelined by qi ----
            def attn_qi(qi):
                L = 2 * qi + 2
                for hhalf in range(2):
                    yall = psy.tile([65, 8, 128], F32, tag="yall")
                    for hh in range(8):
                        h = 8 * hhalf + hh
                        t4, h4 = h // 4, h % 4
                        sps = pss.tile([128, 16, 128], F32, tag="sps")
                        for m in range(L):
                            masked = qi >= 1 and m >= 2 * qi
                            nc.tensor.matmul(
                                sps[:, m, :],
                                kst[t4][32 * h4:32 * h4 + 32, :, 128 * m:128 * (m + 1)],
                                qst[t4][32 * h4:32 * h4 + 32, :, qi, :],
                                start=True, stop=not masked, perf_mode=DRM,
                                tile_position=(32 * h4, 0),
                                skip_group_check=True)
                            if masked:
                                slot = 2 * (qi - 1) + (m - 2 * qi)
                                nc.tensor.matmul(
                                    sps[:, m, :], mneg_sb[:, slot, :], i128_sb,
                                    start=False, stop=True, skip_group_check=True)
                        se = se_p.tile([128, 16, 128], FP8, tag="se")
                        nc.scalar.activation(
                            out=se[:, 0:L, :], in_=sps[:, 0:L, :],
                            func=AF.Exp, scale=float(SCALE))
                        for g in range(L // 2):
                            nc.tensor.matmul(
                                yall[0:65, hh, :],
                                v_ext[:, 2 * g:2 * g + 2, h, 0:65],
                                se[:, 2 * g:2 * g + 2, :],
                                start=(g == 0 and hh % 4 == 0),
                                stop=(g == L // 2 - 1 and hh % 4 == 3),
                                perf_mode=DRM, skip_group_check=True)
                    # normalize 8 heads at once
                    rs = sml_p.tile([1, 1024], BF16, tag="rs")
                    nc.vector.reciprocal(out=rs, in_=yall[64:65, :, :].rearrange(
                        "p a b -> p (a b)"))
                    ri = 2 * qi + hhalf
                    nc.sync.dma_start(out=rs_d[ri][None, :], in_=rs)
                    rb = sml_p.tile([64, 1024], BF16, tag="rb")
                    nc.sync.dma_start(out=rb, in_=bass.AP(
                        tensor=rs_d[ri][None, :].tensor,
                        offset=1024 * ri, ap=[[0, 64], [1, 1024]]))
                    # heads in this half viewed [pair(4), odd(2)]:
                    yv = yall.rearrange("p (a o) b -> p a o b", o=2)
                    rv = rb.rearrange("p (a o b) -> p a o b", a=4, o=2)
                    # evens -> yT[0:64], odds -> staged tile then DMA to yT[64:128]
                    nc.vector.tensor_mul(
                        yT[0:64, 4 * hhalf:4 * hhalf + 4, 128 * qi:128 * (qi + 1)],
                        yv[0:64, :, 0, :], rv[:, :, 0, :])
                    yo = sml_p.tile([64, 4, 128], FP8, tag="yo")
                    nc.vector.tensor_mul(yo, yv[0:64, :, 1, :], rv[:, :, 1, :])
                    nc.sync.dma_start(
                        out=yT[64:128, 4 * hhalf:4 * hhalf + 4,
                               128 * qi:128 * (qi + 1)],
                        in_=yo)

            def proj_t(t):
                for co in range(8):
                    pps = ps1.tile([128, 512], F32, tag="pmid")
                    for half in range(2):
                        qs = slice(512 * t + 256 * half, 512 * t + 256 * (half + 1))
                        for g in range(4):
                            nc.tensor.matmul(
                                pps[:, 256 * half:256 * (half + 1)],
                                wp_sb[:, co, g, :, :], yT[:, 2 * g:2 * g + 2, qs],
                                start=(g == 0), stop=(g == 3), perf_mode=DRM,
                                skip_group_check=True)
                    xq = xin_p.tile([128, 512], F32, tag="xq")
                    nc.sync.dma_start(out=xq, in_=bass.AP(
                        tensor=xT[:, :].tensor,
                        offset=(128 * co) * T + 128 * (8 * t + 1),
                        ap=[[T, 128], [256, 4], [1, 128]]))
                    s_ = xin_p.tile([128, 512], F32, tag="s_")
                    nc.vector.scalar_tensor_tensor(
                        out=s_, in0=pps, scalar=WSI, in1=xq,
                        op0=ALU.mult, op1=ALU.add)
                    nc.gpsimd.tensor_scalar_add(
                        x2[:, co, 512 * t:512 * (t + 1)], s_, cvap("bp", co))
                    if debug:
                        dt_ = out_p.tile([128, 512], F32, tag="dbg")
                        nc.vector.tensor_scalar_add(dt_, s_, cvap("bp", co))
                        nc.sync.dma_start(
                            out=dbg["x2T"][128 * co:128 * (co + 1),
                                           512 * t:512 * (t + 1)],
                            in_=dt_)

            def mlp_t(t):
                gA = big.tile([128, 32, 512], FP8, tag="A")  # reuses hT slot
                for f in range(32):
                    w1_sl = w1s_p.tile([128, 4, 2, 128], FP8, tag="w1")
                    nc.sync.dma_start(out=w1_sl, in_=w1[:, f, :, :, :])
                    fps = ps1.tile([128, 512], F32, tag="pmid")
                    for half in range(2):
                        qs = slice(512 * t + 256 * half, 512 * t + 256 * (half + 1))
                        for g in range(4):
                            nc.tensor.matmul(
                                fps[:, 256 * half:256 * (half + 1)],
                                w1_sl[:, g, :, :], h2T[:, 2 * g:2 * g + 2, qs],
                                start=(g == 0), stop=(g == 3), perf_mode=DRM,
                                skip_group_check=True)
                    nc.scalar.activation(
                        out=gA[:, f, :], in_=fps, func=AF.Gelu,
                        bias=cvap("b1", f), scale=WSI)
                for co in range(8):
                    ops_ = ps1.tile([128, 512], F32, tag="pmid")
                    w2_sl = w2s_p.tile([128, 16, 2, 128], FP8, tag="w2")
                    nc.sync.dma_start(out=w2_sl, in_=w2[:, co, :, :, :])
                    for half in range(2):
                        for g in range(16):
                            nc.tensor.matmul(
                                ops_[:, 256 * half:256 * (half + 1)],
                                w2_sl[:, g, :, :],
                                gA[:, 2 * g:2 * g + 2,
                                   256 * half:256 * (half + 1)],
                                start=(g == 0), stop=(g == 15), perf_mode=DRM,
                                skip_group_check=True)
                    o1 = xin_p.tile([128, 512], F32, tag="s_")
                    nc.vector.scalar_tensor_tensor(
                        out=o1, in0=ops_, scalar=WSI,
                        in1=x2[:, co, 512 * t:512 * (t + 1)],
                        op0=ALU.mult, op1=ALU.add)
                    o2 = out_p.tile([128, 512], F32, tag="o2")
                    nc.gpsimd.tensor_scalar_add(o2, o1, cvap("b2", co))
                    nc.sync.dma_start(
                        out=outT[128 * co:128 * (co + 1), 512 * t:512 * (t + 1)],
                        in_=o2)

            for qi in range(4):
                attn_qi(qi)
            attn_qi(4)
            proj_t(0)
            attn_qi(5)
            ln(None, x2, 0, 512, T, h2T, "g2", "bt2")
            attn_qi(6)
            mlp_t(0)
            attn_qi(7)
            proj_t(1)
            ln(None, x2, 512, 512, T, h2T, "g2", "bt2")

            if debug:
                for p in range(8):
                    for t in range(2):
                        dt_ = out_p.tile([128, 512], F32, tag="dbg")
                        nc.vector.tensor_copy(
                            out=dt_, in_=yT[:, p, 512 * t:512 * (t + 1)])
                        nc.sync.dma_start(
                            out=dbg["yT"][128 * p:128 * (p + 1),
                                          512 * t:512 * (t + 1)],
                            in_=dt_)

            mlp_t(1)

    return nc


# ===================== host side =====================

def make_mneg(rho, cond):
    """14 slots [(qi,ms) for qi in 1..7 for ms in 0,1], each stored transposed
    [q, k] fp8: 0 where allowed, -240 where masked. Key chunk at permuted
    position m corresponds to natural chunk (pairswap(m) if rho==0 else m)."""
    out = np.zeros((14, 128, 128), dtype=np.float32)
    for qi in range(1, 8):
        qnat = 2 * qi + rho          # natural q chunk
        qrows = 128 * qnat + np.arange(128)
        for ms in range(2):
            m = 2 * qi + ms          # permuted key-chunk position
            n = (m ^ 1) if rho == 0 else m   # natural key chunk
            krows = 128 * n + np.arange(128)
            i = qrows[None, :]       # [1, q] -> transposed layout [q, k] idx [k, q]?
            # allowed(i=query, j=key); stored slot[q, k]
            qq = qrows[:, None]
            kk = krows[None, :]
            allowed = (kk < cond) | ((qq >= cond) & (kk >= cond) & (kk <= qq))
            out[2 * (qi - 1) + ms] = np.where(allowed, 0.0, MNEG)
    return out.reshape(14 * 128, 128).astype(E4)


def pack_dr_w(W, gdim, fblk):
    """W [1024 or 4096, F] -> [128, nf, g, 2, fblk] with c = 256g+128i+p."""
    Cin, F = W.shape
    ng = Cin // 256
    nf = F // fblk
    out = np.empty((128, nf, ng, 2, fblk), dtype=E4)
    Ws = (W * WS).astype(E4)
    for fi in range(nf):
        blk = Ws[:, fblk * fi:fblk * (fi + 1)]          # [Cin, fblk]
        r = blk.reshape(ng, 2, 128, fblk)                # g, i, p, f
        out[:, fi] = r.transpose(2, 0, 1, 3)             # p, g, i, f
    return np.ascontiguousarray(out)


def perm_cols(rho):
    """column permutation of T at 128 granularity: rho=0 pairswap, rho=1 id."""
    chunks = np.arange(16)
    if rho == 0:
        chunks = chunks ^ 1
    idx = (128 * chunks[:, None] + np.arange(128)[None, :]).reshape(-1)
    return idx


def shard_inputs(inputs):
    x = np.asarray(inputs["x"], np.float32)
    cond = int(np.asarray(inputs["cond_len"]))
    cvec = np.zeros((128, 88), np.float32)
    for name, key in [("bq", "bq"), ("bp", "bp"), ("b2", "b2"), ("g1", "ln1_g"),
                      ("bt1", "ln1_b"), ("g2", "ln2_g"), ("bt2", "ln2_b")]:
        v = np.asarray(inputs[key], np.float32)
        cvec[:, CV[name]:CV[name] + 8] = v.reshape(8, 128).T
    cvec[:, CV["b1"]:CV["b1"] + 32] = np.asarray(
        inputs["b1"], np.float32).reshape(32, 128).T
    common = {
        "wq": pack_dr_w(np.asarray(inputs["Wq"], np.float32), 4, 128),
        "wk": pack_dr_w(np.asarray(inputs["Wk"], np.float32), 4, 128),
        "wv": pack_dr_w(np.asarray(inputs["Wv"], np.float32), 4, 256),
        "wp": pack_dr_w(np.asarray(inputs["Wp"], np.float32), 4, 128),
        "w1": pack_dr_w(np.asarray(inputs["W1"], np.float32), 4, 128),
        "w2": pack_dr_w(np.asarray(inputs["W2"], np.float32), 16, 128),
        "cvec": cvec,
        "bvb": np.ascontiguousarray(np.broadcast_to(
            np.asarray(inputs["bv"], np.float32), (128, C))).astype(
                ml_dtypes.bfloat16),
        "i128": np.eye(128, dtype=np.float32).astype(E4),
    }
    in_maps, row_sets = [], []
    for c in range(8):
        b, rho = c // 2, c % 2
        m = dict(common)
        cols = perm_cols(rho)
        m["xT"] = np.ascontiguousarray(x[b].T[:, cols])
        m["mneg"] = make_mneg(rho, cond)
        rows = np.concatenate(
            [np.arange(128 * (2 * qi + rho), 128 * (2 * qi + rho) + 128)
             for qi in range(8)])
        row_sets.append((b, rows))
        in_maps.append(m)
    return in_maps, row_sets


_cached_nc = {}


def get_nc(debug=False):
    if debug not in _cached_nc:
        _cached_nc[debug] = build_nc(debug=debug)
    return _cached_nc[debug]


def run(inputs, debug=False):
    nc = get_nc(debug=debug)
    in_maps, row_sets = shard_inputs(inputs)
    res = run_bass_kernel_spmd(nc, in_maps, core_ids=list(range(8)))
    x = np.asarray(inputs["x"], np.float32)
    out = np.empty_like(x)
    for c in range(8):
        b, rows = row_sets[c]
        out[b][rows] = res.results[c]["outT"].T
    return out, res, row_sets


def kernel(**inputs):
    out, _, _ = run(inputs, debug=False)
    return out


# revision 3
# speedup vs baseline: 1.0131x; 1.0131x over previous
"""Trainium2 kernel v2 for the dense transformer block (B=4, T=2048, C=1024,
H=16, MLP 4x, hybrid cond/causal mask), SPMD over 8 cores, collective-free.

Core (b, rho) = (core//2, core%2) handles batch b, query chunks {2*qi+rho}.
The T axis is chunk-permuted per core (pair-swap for rho=0) so ONE program
(query chunk at position 2*qi+1, kv limit 2*qi+2) serves both rho values.
All GEMMs run in fp8e4m3 DoubleRow (weights scaled 2^12 host-side); masks are
added as -240 bias via PE identity-matmuls before the softmax exp.
"""
import sys
sys.path.insert(0, '/opt/trn_rl_repo')
import numpy as np
import ml_dtypes
import concourse.bass as bass
import concourse.mybir as mybir
import concourse.tile as tile
from concourse.vector_clock import ScopedClock
from concourse.bass_utils import run_bass_kernel_spmd

# ---- walrus 1-sync-wait-per-instruction workarounds (from baseline) ----
_installed = False


def _split_multi_waits(ordered_by_block, nc):
    for bb_name, insts in ordered_by_block.items():
        need = any(
            inst.sync_info is not None and len(inst.sync_info.on_wait) > 1
            for inst in insts
        )
        if not need:
            continue
        new_list = []
        for inst in insts:
            si = inst.sync_info
            waits = list(si.on_wait) if si is not None and si.on_wait else []
            if len(waits) > 1:
                for w in waits[:-1]:
                    nop = mybir.InstNoOp(
                        name=nc.get_next_instruction_name(),
                        ins=[],
                        outs=[],
                        bass_nofuse=True,
                    )
                    nop.engine = inst.engine
                    nop.sync_info = mybir.SyncInfo(on_wait=[w], on_update=[])
                    new_list.append(nop)
                ups = list(si.on_update) if si.on_update else []
                inst.sync_info = mybir.SyncInfo(on_wait=[waits[-1]], on_update=ups)
            new_list.append(inst)
        insts[:] = new_list


class _SplitWaitClockWait:
    def __init__(self, tc, ordered, **kw):
        import bass_rust
        self._inner = bass_rust.TileClockWait(tc, ordered, **kw)
        self._tc = tc
        self._ordered = ordered

    def __getattr__(self, a):
        return getattr(self._inner, a)

    def assign_waits(self, bb_name):
        r = self._inner.assign_waits(bb_name)
        _split_multi_waits(self._ordered, self._tc.nc)
        return r


class PatchedTileContext(tile.TileContext):
    """TileContext whose final drain carries at most one sem wait."""

    def _drain_and_barrier(self, tick_clock, wait_clock):
        probe = self.nc.sync.nop(nofuse=True)
        add = wait_clock.add_sem_waits
        add(probe.ins, ScopedClock({None: tick_clock.global_clock}))
        si = probe.ins.sync_info
        waits = list(si.on_wait) if si is not None and si.on_wait else []
        if len(waits) > 1:
            probe.ins.sync_info = mybir.SyncInfo(on_wait=[waits[0]], on_update=[])
            for w in waits[1:]:
                n = self.nc.sync.nop(nofuse=True)
                n.ins.sync_info = mybir.SyncInfo(on_wait=[w], on_update=[])
        self.nc.sync.drain()
        self.nc.all_engine_barrier()
        popped = self.nc._tile_sem_poison_stack.pop()
        assert popped is self._sem_poison
        self.nc.clear_and_free_semaphores(list(self.sems.allocated().values()))
        self.nc.all_engine_barrier()


def _install():
    global _installed
    if not _installed:
        tile.TileClockWait = _SplitWaitClockWait
        _installed = True


_install()

# ---- constants ----

F32 = mybir.dt.float32
BF16 = mybir.dt.bfloat16
FP8 = mybir.dt.float8e4
E4 = ml_dtypes.float8_e4m3
AF = mybir.ActivationFunctionType
ALU = mybir.AluOpType
DRM = mybir.MatmulPerfMode.DoubleRow

C = 1024
T = 2048
H = 16
HD = 64
FF = 4096
COND = 256
EPS = 1e-5
SCALE = 1.0 / np.sqrt(HD)
WS = 2.0 ** 12       # weight scale
WSI = float(2.0 ** -12)
MNEG = -240.0
NQI = 8              # query chunks per core
# cvec column indices in the packed [128, 88] bias tensor
CV = {"bq": 0, "bp": 8, "b2": 16, "g1": 24, "bt1": 32, "g2": 40, "bt2": 48, "b1": 56}


def build_nc(debug=False):
    nc = bass.Bass()
    xT = nc.dram_tensor("xT", [C, T], F32, kind="ExternalInput")
    wq = nc.dram_tensor("wq", [128, 8, 4, 2, 128], FP8, kind="ExternalInput")
    wk = nc.dram_tensor("wk", [128, 8, 4, 2, 128], FP8, kind="ExternalInput")
    wv = nc.dram_tensor("wv", [128, 4, 4, 2, 256], FP8, kind="ExternalInput")
    wp = nc.dram_tensor("wp", [128, 8, 4, 2, 128], FP8, kind="ExternalInput")
    w1 = nc.dram_tensor("w1", [128, 32, 4, 2, 128], FP8, kind="ExternalInput")
    w2 = nc.dram_tensor("w2", [128, 8, 16, 2, 128], FP8, kind="ExternalInput")
    cvec = nc.dram_tensor("cvec", [128, 88], F32, kind="ExternalInput")
    bvb = nc.dram_tensor("bvb", [128, C], BF16, kind="ExternalInput")
    mneg = nc.dram_tensor("mneg", [14 * 128, 128], FP8, kind="ExternalInput")
    i128 = nc.dram_tensor("i128", [128, 128], FP8, kind="ExternalInput")
    outT = nc.dram_tensor("outT", [C, 1024], F32, kind="ExternalOutput")

    stats_d = nc.dram_tensor("stats_d", [2, T + 1024], BF16)   # a=rstd rows, b=-mu*rstd

    dbg = {}
    if debug:
        dbg["hT"] = nc.dram_tensor("dbg_hT", [C, T], F32, kind="ExternalOutput")
        dbg["v"] = nc.dram_tensor("dbg_v", [T, H * HD], F32, kind="ExternalOutput")
        dbg["yT"] = nc.dram_tensor("dbg_yT", [C, 1024], F32, kind="ExternalOutput")
        dbg["x2T"] = nc.dram_tensor("dbg_x2T", [C, 1024], F32, kind="ExternalOutput")

    with PatchedTileContext(nc) as tc:
        import contextlib
        with contextlib.ExitStack() as _es:
            _e = _es.enter_context
            _e(nc.allow_low_precision(reason="bf16/fp8 intermediates validated vs numpy"))
            big = _e(tc.tile_pool(name="big", bufs=1))
            wbig = _e(tc.tile_pool(name="wbig", bufs=1))
            w1s_p = _e(tc.tile_pool(name="w1s", bufs=2))
            w2s_p = _e(tc.tile_pool(name="w2s", bufs=2))
            st = _e(tc.tile_pool(name="st", bufs=1))
            xin_p = _e(tc.tile_pool(name="xin", bufs=2))
            xb_p = _e(tc.tile_pool(name="xb", bufs=2))
            bc_p = _e(tc.tile_pool(name="bc", bufs=2))
            rows_p = _e(tc.tile_pool(name="rows", bufs=1))
            se_p = _e(tc.tile_pool(name="se", bufs=2))
            ktmp_p = _e(tc.tile_pool(name="ktmp", bufs=2))
            sml_p = _e(tc.tile_pool(name="sml", bufs=2))
            out_p = _e(tc.tile_pool(name="outp", bufs=2))
            pss = _e(tc.tile_pool(name="pss", bufs=2, space="PSUM"))
            psy = _e(tc.tile_pool(name="psy", bufs=1, space="PSUM"))
            ps1 = _e(tc.tile_pool(name="ps1", bufs=2, space="PSUM"))
            # ---------------- resident tensors ----------------
            hT = big.tile([128, 8, T], FP8, tag="A")
            v_ext = big.tile([128, 16, 16, 80], FP8, tag="V")
            kst = [big.tile([128, 2, T], FP8, tag=f"K{t4}", name=f"kst{t4}") for t4 in range(4)]
            qst = [big.tile([128, 2, NQI, 128], FP8, tag=f"Q{t4}", name=f"qst{t4}") for t4 in range(4)]
            yT = big.tile([128, 8, 1024], FP8, tag="Y")
            x2 = big.tile([128, 8, 1024], BF16, tag="X2")
            h2T = big.tile([128, 8, 1024], FP8, tag="H2")
            wq_sb = wbig.tile([128, 8, 4, 2, 128], FP8, tag="WQ")
            wk_sb = wbig.tile([128, 8, 4, 2, 128], FP8, tag="WK")
            wv_sb = wbig.tile([128, 4, 4, 2, 256], FP8, tag="WV")
            wp_sb = wbig.tile([128, 8, 4, 2, 128], FP8, tag="WP")
            cv = st.tile([128, 88], F32)
            bvb_sb = st.tile([128, C], BF16)
            mneg_sb = st.tile([128, 14, 128], FP8)
            i128_sb = st.tile([128, 128], FP8)
            ones1 = st.tile([128, 1], BF16)
            ones64 = st.tile([1, 64], BF16)
            eps1 = st.tile([1, 1], F32)

            nc.vector.memset(ones1, 1.0)
            nc.vector.memset(ones64, 1.0)
            nc.vector.memset(eps1, EPS)
            nc.sync.dma_start(out=wq_sb, in_=wq[:, :, :, :, :])
            nc.sync.dma_start(out=wk_sb, in_=wk[:, :, :, :, :])
            nc.sync.dma_start(out=wv_sb, in_=wv[:, :, :, :, :])
            nc.sync.dma_start(out=wp_sb, in_=wp[:, :, :, :, :])
            nc.sync.dma_start(out=cv, in_=cvec[:, :])
            nc.sync.dma_start(out=bvb_sb, in_=bvb[:, :])
            nc.sync.dma_start(out=mneg_sb, in_=mneg.rearrange("(s p) k -> p s k", p=128))
            nc.sync.dma_start(out=i128_sb, in_=i128[:, :])

            def cvap(name, i):
                return cv[:, CV[name] + i:CV[name] + i + 1]

            # ---------------- layernorm (x source: dram f32 or sbuf bf16) ------
            def ln(src_dram, src_sb, col0, ncols, stat_off, out_tile, gname, btname,
                   after_tile=None, sq_on_act=False):
                """src [C, *]; normalizes cols [col0, col0+ncols) writing fp8
                (affine g/bt) into out_tile[:, cc, same cols]. Stats broadcast
                via stats_d rows at stat_off+col."""
                nt = ncols // 512
                for t in range(nt):
                    cs = slice(col0 + 512 * t, col0 + 512 * (t + 1))
                    stat_ps = pss.tile([33, 512], F32, tag="sps")
                    mu_ps = stat_ps[0:1, :]
                    sq_ps = stat_ps[32:33, :]
                    xbt = xb_p.tile([128, 8, 512], BF16, tag="xb")
                    for cc in range(8):
                        if src_dram is not None:
                            xt = xin_p.tile([128, 512], F32, tag="x")
                            nc.sync.dma_start(
                                out=xt, in_=src_dram[128 * cc:128 * (cc + 1), cs])
                            nc.vector.tensor_copy(out=xbt[:, cc, :], in_=xt)
                        else:
                            nc.vector.tensor_copy(
                                out=xbt[:, cc, :], in_=src_sb[:, cc, cs])
                        sq = xin_p.tile([128, 512], BF16, tag="sq")
                        if sq_on_act:
                            nc.scalar.activation(out=sq, in_=xbt[:, cc, :],
                                                 func=AF.Square)
                        else:
                            nc.vector.tensor_mul(sq, xbt[:, cc, :], xbt[:, cc, :])
                        nc.tensor.matmul(mu_ps, ones1, xbt[:, cc, :],
                                         start=(cc == 0), stop=(cc == 7))
                        nc.tensor.matmul(sq_ps, ones1, sq,
                                         start=(cc == 0), stop=(cc == 7))
                    mu = rows_p.tile([1, 512], F32, tag="mu")
                    ex2 = rows_p.tile([1, 512], F32, tag="t2")
                    nc.vector.tensor_scalar_mul(mu, mu_ps, 1.0 / C)
                    nc.vector.tensor_scalar_mul(ex2, sq_ps, 1.0 / C)
                    var = rows_p.tile([1, 512], F32, tag="t3")
                    nc.vector.scalar_tensor_tensor(
                        out=var, in0=mu, scalar=-1.0, in1=mu, op0=ALU.mult, op1=ALU.mult)
                    nc.vector.tensor_add(var, var, ex2)
                    std = rows_p.tile([1, 512], F32, tag="t2")
                    nc.scalar.activation(out=std, in_=var, func=AF.Sqrt, bias=eps1)
                    arow = rows_p.tile([1, 512], BF16, tag="ab16")
                    nc.vector.reciprocal(out=arow, in_=std)
                    brow = rows_p.tile([1, 512], BF16, tag="ab16b")
                    nc.vector.scalar_tensor_tensor(
                        out=brow, in0=mu, scalar=-1.0, in1=arow,
                        op0=ALU.mult, op1=ALU.mult)
                    so = stat_off + col0 + 512 * t
                    nc.sync.dma_start(out=stats_d[0, so:so + 512][None, :], in_=arow)
                    nc.sync.dma_start(out=stats_d[1, so:so + 512][None, :], in_=brow)
                    a_b = bc_p.tile([128, 512], BF16, tag="a_b")
                    b_b = bc_p.tile([128, 512], BF16, tag="b_b")
                    nc.sync.dma_start(out=a_b, in_=bass.AP(
                        tensor=stats_d[0][None, :].tensor,
                        offset=so, ap=[[0, 128], [1, 512]]))
                    nc.sync.dma_start(out=b_b, in_=bass.AP(
                        tensor=stats_d[1][None, :].tensor,
                        offset=(T + 1024) + so, ap=[[0, 128], [1, 512]]))
                    for cc in range(8):
                        t1 = xin_p.tile([128, 512], BF16, tag="t1")
                        nc.vector.tensor_mul(t1, xbt[:, cc, :], a_b)
                        u = xin_p.tile([128, 512], BF16, tag="u")
                        nc.vector.tensor_add(u, t1, b_b)
                        nc.gpsimd.tensor_scalar(
                            out_tile[:, cc, cs], u, cvap(gname, cc), cvap(btname, cc),
                            op0=ALU.mult, op1=ALU.add)
                    if after_tile is not None:
                        after_tile(t)

            def emit_v(m):
                for fh in range(4):
                    vps = ps1.tile([128, 256], F32, tag="pmid")
                    for g in range(4):
                        nc.tensor.matmul(
                            vps, hT[:, 2 * g:2 * g + 2, 128 * m:128 * (m + 1)],
                            wv_sb[:, fh, g, :, :],
                            start=(g == 0), stop=(g == 3), perf_mode=DRM,
                            skip_group_check=True)
                    nc.vector.scalar_tensor_tensor(
                        out=v_ext[:, m, 4 * fh:4 * fh + 4, 0:HD],
                        in0=vps.rearrange("p (h d) -> p h d", h=4), scalar=WSI,
                        in1=bvb_sb[:, 256 * fh:256 * (fh + 1)].rearrange(
                            "p (h d) -> p h d", h=4),
                        op0=ALU.mult, op1=ALU.add)
                nc.vector.memset(v_ext[:, m, :, HD:HD + 1], 1.0)

            def emit_qh(qh):
                for p in range(8):
                    qtmp = ktmp_p.tile([128, 4, 128], FP8, tag="qt")
                    qps = ps1.tile([128, 4, 128], F32, tag="pmid")
                    for qi4 in range(4):
                        qi = 4 * qh + qi4
                        qc0 = 128 * (2 * qi + 1)
                        for g in range(4):
                            nc.tensor.matmul(
                                qps[:, qi4, :], wq_sb[:, p, g, :, :],
                                hT[:, 2 * g:2 * g + 2, qc0:qc0 + 128],
                                start=(g == 0), stop=(g == 3), perf_mode=DRM,
                                skip_group_check=True)
                    nc.vector.tensor_scalar(
                        qtmp.rearrange("p a b -> p (a b)"),
                        qps.rearrange("p a b -> p (a b)"),
                        WSI, cvap("bq", p), op0=ALU.mult, op1=ALU.add)
                    for odd in range(2):
                        h = 2 * p + odd
                        t4, h4 = h // 4, h % 4
                        for i in range(2):
                            nc.sync.dma_start(
                                out=qst[t4][32 * h4:32 * h4 + 32, i,
                                            4 * qh:4 * qh + 4, :],
                                in_=qtmp[64 * odd + 32 * i:64 * odd + 32 * i + 32,
                                         :, :])

            def ln1_after(t):
                for m in range(4 * t, 4 * t + 4):
                    emit_v(m)
                if t == 1:
                    emit_qh(0)
                if t == 3:
                    emit_qh(1)

            ln(xT, None, 0, T, 0, hT, "g1", "bt1", after_tile=ln1_after, sq_on_act=True)

            if debug:
                for cc in range(8):
                    for t in range(4):
                        dt_ = out_p.tile([128, 512], F32, tag="dbg")
                        nc.vector.tensor_copy(out=dt_, in_=hT[:, cc, 512 * t:512 * (t + 1)])
                        nc.sync.dma_start(
                            out=dbg["hT"][128 * cc:128 * (cc + 1), 512 * t:512 * (t + 1)],
                            in_=dt_)


            if debug:
                for m in range(16):
                    dt_ = out_p.tile([128, H * HD], F32, tag="dbgv")
                    nc.vector.tensor_copy(
                        out=dt_.rearrange("p (h d) -> p h d", h=H),
                        in_=v_ext[:, m, :, 0:HD])
                    nc.sync.dma_start(out=dbg["v"][128 * m:128 * (m + 1), :], in_=dt_)

            # ---------------- K (then remap to kst) ----------------
            for p in range(8):
                ktmp = ktmp_p.tile([128, 2048], FP8, tag="kt")
                for t in range(4):
                    kps = ps1.tile([128, 512], F32, tag="pmid")
                    for half in range(2):
                        cs = slice(512 * t + 256 * half, 512 * t + 256 * (half + 1))
                        for g in range(4):
                            nc.tensor.matmul(
                                kps[:, 256 * half:256 * (half + 1)],
                                wk_sb[:, p, g, :, :], hT[:, 2 * g:2 * g + 2, cs],
                                start=(g == 0), stop=(g == 3), perf_mode=DRM,
                                skip_group_check=True)
                    nc.vector.tensor_scalar_mul(
                        ktmp[:, 512 * t:512 * (t + 1)], kps, WSI)
                for odd in range(2):
                    h = 2 * p + odd
                    t4, h4 = h // 4, h % 4
                    for i in range(2):
                        nc.sync.dma_start(
                            out=kst[t4][32 * h4:32 * h4 + 32, i, :],
                            in_=ktmp[64 * odd + 32 * i:64 * odd + 32 * i + 32, :])

            # ---------------- attention + proj + LN2 + MLP, pipelined by qi ----
            def attn_qi(qi):
                L = 2 * qi + 2
                for hhalf in range(2):
                    yall = psy.tile([65, 8, 128], F32, tag="yall")
                    for hh in range(8):
                        h = 8 * hhalf + hh
                        t4, h4 = h // 4, h % 4
                        se = se_p.tile([128, 16, 128], FP8, tag="se")
                        for off in range(0, L, 8):
                            pl = min(8, L - off)
                            sps = pss.tile([128, 8, 128], F32, tag="sps")
                            for mi in range(pl):
                                m = off + mi
                                masked = qi >= 1 and m >= 2 * qi
                                nc.tensor.matmul(
                                    sps[:, mi, :],
                                    kst[t4][32 * h4:32 * h4 + 32, :,
                                            128 * m:128 * (m + 1)],
                                    qst[t4][32 * h4:32 * h4 + 32, :, qi, :],
                                    start=True, stop=not masked, perf_mode=DRM,
                                    tile_position=(32 * h4, 0),
                                    skip_group_check=True)
                                if masked:
                                    slot = 2 * (qi - 1) + (m - 2 * qi)
                                    nc.tensor.matmul(
                                        sps[:, mi, :], mneg_sb[:, slot, :], i128_sb,
                                        start=False, stop=True,
                                        skip_group_check=True)
                            nc.scalar.activation(
                                out=se[:, off:off + pl, :], in_=sps[:, 0:pl, :],
                                func=AF.Exp, scale=float(SCALE))
                        for g in range(L // 2):
                            nc.tensor.matmul(
                                yall[0:65, hh, :],
                                v_ext[:, 2 * g:2 * g + 2, h, 0:65],
                                se[:, 2 * g:2 * g + 2, :],
                                start=(g == 0 and hh % 4 == 0),
                                stop=(g == L // 2 - 1 and hh % 4 == 3),
                                perf_mode=DRM, skip_group_check=True)
                    # normalize 8 heads at once; free yall (PSUM) early by
                    # staging sums+values to SBUF, broadcast recips via PE
                    rs = sml_p.tile([1, 1024], BF16, tag="rs")
                    nc.vector.reciprocal(
                        out=rs.rearrange("p (o a b) -> p o a b", o=2, a=4),
                        in_=yall[64:65, :, :].rearrange("p (a o) b -> p o a b",
                                                        o=2))
                    ysb = sml_p.tile([64, 8, 128], BF16, tag="ysb")
                    nc.vector.tensor_copy(out=ysb, in_=yall[0:64, :, :])
                    yv = ysb.rearrange("p (a o) b -> p a o b", o=2)
                    for o in range(2):
                        rbps = ps1.tile([64, 512], F32, tag="pmid")
                        nc.tensor.matmul(rbps, ones64, rs[:, 512 * o:512 * (o + 1)],
                                         start=True, stop=True)
                        rv = rbps.rearrange("p (a b) -> p a b", a=4)
                        if o == 0:
                            nc.vector.tensor_mul(
                                yT[0:64, 4 * hhalf:4 * hhalf + 4,
                                   128 * qi:128 * (qi + 1)],
                                yv[:, :, 0, :], rv)
                        else:
                            yo = sml_p.tile([64, 4, 128], FP8, tag="yo")
                            nc.vector.tensor_mul(yo, yv[:, :, 1, :], rv)
                            nc.sync.dma_start(
                                out=yT[64:128, 4 * hhalf:4 * hhalf + 4,
                                       128 * qi:128 * (qi + 1)],
                                in_=yo)

            def proj_t(t):
                for co in range(8):
                    pps = ps1.tile([128, 512], F32, tag="pmid")
                    for half in range(2):
                        qs = slice(512 * t + 256 * half, 512 * t + 256 * (half + 1))
                        for g in range(4):
                            nc.tensor.matmul(
                                pps[:, 256 * half:256 * (half + 1)],
                                wp_sb[:, co, g, :, :], yT[:, 2 * g:2 * g + 2, qs],
                                start=(g == 0), stop=(g == 3), perf_mode=DRM,
                                skip_group_check=True)
                    xq = xin_p.tile([128, 512], F32, tag="xq")
                    nc.sync.dma_start(out=xq, in_=bass.AP(
                        tensor=xT[:, :].tensor,
                        offset=(128 * co) * T + 128 * (8 * t + 1),
                        ap=[[T, 128], [256, 4], [1, 128]]))
                    s_ = xin_p.tile([128, 512], F32, tag="s_")
                    nc.vector.scalar_tensor_tensor(
                        out=s_, in0=pps, scalar=WSI, in1=xq,
                        op0=ALU.mult, op1=ALU.add)
                    nc.gpsimd.tensor_scalar_add(
                        x2[:, co, 512 * t:512 * (t + 1)], s_, cvap("bp", co))
                    if debug:
                        dt_ = out_p.tile([128, 512], F32, tag="dbg")
                        nc.vector.tensor_scalar_add(dt_, s_, cvap("bp", co))
                        nc.sync.dma_start(
                            out=dbg["x2T"][128 * co:128 * (co + 1),
                                           512 * t:512 * (t + 1)],
                            in_=dt_)

            def mlp_t(t):
                gA = big.tile([128, 32, 512], FP8, tag="A")  # reuses hT slot
                for f in range(32):
                    w1_sl = w1s_p.tile([128, 4, 2, 128], FP8, tag="w1")
                    nc.sync.dma_start(out=w1_sl, in_=w1[:, f, :, :, :])
                    fps = ps1.tile([128, 512], F32, tag="pmid")
                    for half in range(2):
                        qs = slice(512 * t + 256 * half, 512 * t + 256 * (half + 1))
                        for g in range(4):
                            nc.tensor.matmul(
                                fps[:, 256 * half:256 * (half + 1)],
                                w1_sl[:, g, :, :], h2T[:, 2 * g:2 * g + 2, qs],
                                start=(g == 0), stop=(g == 3), perf_mode=DRM,
                                skip_group_check=True)
                    nc.scalar.activation(
                        out=gA[:, f, :], in_=fps, func=AF.Gelu,
                        bias=cvap("b1", f), scale=WSI)
                for co in range(8):
                    ops_ = ps1.tile([128, 512], F32, tag="pmid")
                    w2_sl = w2s_p.tile([128, 16, 2, 128], FP8, tag="w2")
                    nc.sync.dma_start(out=w2_sl, in_=w2[:, co, :, :, :])
                    for half in range(2):
                        for g in range(16):
                            nc.tensor.matmul(
                                ops_[:, 256 * half:256 * (half + 1)],
                                w2_sl[:, g, :, :],
                                gA[:, 2 * g:2 * g + 2,
                                   256 * half:256 * (half + 1)],
                                start=(g == 0), stop=(g == 15), perf_mode=DRM,
                                skip_group_check=True)
                    o1 = xin_p.tile([128, 512], F32, tag="s_")
                    nc.vector.scalar_tensor_tensor(
                        out=o1, in0=ops_, scalar=WSI,
                        in1=x2[:, co, 512 * t:512 * (t + 1)],
                        op0=ALU.mult, op1=ALU.add)
                    o2 = out_p.tile([128, 512], F32, tag="o2")
                    nc.gpsimd.tensor_scalar_add(o2, o1, cvap("b2", co))
                    nc.sync.dma_start(
                        out=outT[128 * co:128 * (co + 1), 512 * t:512 * (t + 1)],
                        in_=o2)

            for qi in range(4):
                attn_qi(qi)
            attn_qi(4)
            proj_t(0)
            attn_qi(5)
            ln(None, x2, 0, 512, T, h2T, "g2", "bt2")
            attn_qi(6)
            mlp_t(0)
            attn_qi(7)
            proj_t(1)
            ln(None, x2, 512, 512, T, h2T, "g2", "bt2")

            if debug:
                for p in range(8):
                    for t in range(2):
                        dt_ = out_p.tile([128, 512], F32, tag="dbg")
                        nc.vector.tensor_copy(
                            out=dt_, in_=yT[:, p, 512 * t:512 * (t + 1)])
                        nc.sync.dma_start(
                            out=dbg["yT"][128 * p:128 * (p + 1),
                                          512 * t:512 * (t + 1)],
                            in_=dt_)

            mlp_t(1)

    return nc


# ===================== host side =====================

def make_mneg(rho, cond):
    """14 slots [(qi,ms) for qi in 1..7 for ms in 0,1], each stored transposed
    [q, k] fp8: 0 where allowed, -240 where masked. Key chunk at permuted
    position m corresponds to natural chunk (pairswap(m) if rho==0 else m)."""
    out = np.zeros((14, 128, 128), dtype=np.float32)
    for qi in range(1, 8):
        qnat = 2 * qi + rho          # natural q chunk
        qrows = 128 * qnat + np.arange(128)
        for ms in range(2):
            m = 2 * qi + ms          # permuted key-chunk position
            n = (m ^ 1) if rho == 0 else m   # natural key chunk
            krows = 128 * n + np.arange(128)
            i = qrows[None, :]       # [1, q] -> transposed layout [q, k] idx [k, q]?
            # allowed(i=query, j=key); stored slot[q, k]
            qq = qrows[:, None]
            kk = krows[None, :]
            allowed = (kk < cond) | ((qq >= cond) & (kk >= cond) & (kk <= qq))
            out[2 * (qi - 1) + ms] = np.where(allowed, 0.0, MNEG)
    return out.reshape(14 * 128, 128).astype(E4)


def pack_dr_w(W, gdim, fblk):
    """W [1024 or 4096, F] -> [128, nf, g, 2, fblk] with c = 256g+128i+p."""
    Cin, F = W.shape
    ng = Cin // 256
    nf = F // fblk
    out = np.empty((128, nf, ng, 2, fblk), dtype=E4)
    Ws = (W * WS).astype(E4)
    for fi in range(nf):
        blk = Ws[:, fblk * fi:fblk * (fi + 1)]          # [Cin, fblk]
        r = blk.reshape(ng, 2, 128, fblk)                # g, i, p, f
        out[:, fi] = r.transpose(2, 0, 1, 3)             # p, g, i, f
    return np.ascontiguousarray(out)


def perm_cols(rho):
    """column permutation of T at 128 granularity: rho=0 pairswap, rho=1 id."""
    chunks = np.arange(16)
    if rho == 0:
        chunks = chunks ^ 1
    idx = (128 * chunks[:, None] + np.arange(128)[None, :]).reshape(-1)
    return idx


def shard_inputs(inputs):
    x = np.asarray(inputs["x"], np.float32)
    cond = int(np.asarray(inputs["cond_len"]))
    cvec = np.zeros((128, 88), np.float32)
    for name, key in [("bq", "bq"), ("bp", "bp"), ("b2", "b2"), ("g1", "ln1_g"),
                      ("bt1", "ln1_b"), ("g2", "ln2_g"), ("bt2", "ln2_b")]:
        v = np.asarray(inputs[key], np.float32)
        cvec[:, CV[name]:CV[name] + 8] = v.reshape(8, 128).T
    cvec[:, CV["b1"]:CV["b1"] + 32] = np.asarray(
        inputs["b1"], np.float32).reshape(32, 128).T
    common = {
        "wq": pack_dr_w(np.asarray(inputs["Wq"], np.float32), 4, 128),
        "wk": pack_dr_w(np.asarray(inputs["Wk"], np.float32), 4, 128),
        "wv": pack_dr_w(np.asarray(inputs["Wv"], np.float32), 4, 256),
        "wp": pack_dr_w(np.asarray(inputs["Wp"], np.float32), 4, 128),
        "w1": pack_dr_w(np.asarray(inputs["W1"], np.float32), 4, 128),
        "w2": pack_dr_w(np.asarray(inputs["W2"], np.float32), 16, 128),
        "cvec": cvec,
        "bvb": np.ascontiguousarray(np.broadcast_to(
            np.asarray(inputs["bv"], np.float32), (128, C))).astype(
                ml_dtypes.bfloat16),
        "i128": np.eye(128, dtype=np.float32).astype(E4),
    }
    in_maps, row_sets = [], []
    for c in range(8):
        b, rho = c // 2, c % 2
        m = dict(common)
        cols = perm_cols(rho)
        m["xT"] = np.ascontiguousarray(x[b].T[:, cols])
        m["mneg"] = make_mneg(rho, cond)
        rows = np.concatenate(
            [np.arange(128 * (2 * qi + rho), 128 * (2 * qi + rho) + 128)
             for qi in range(8)])
        row_sets.append((b, rows))
        in_maps.append(m)
    return in_maps, row_sets


_cached_nc = {}


def get_nc(debug=False):
    if debug not in _cached_nc:
        _cached_nc[debug] = build_nc(debug=debug)
    return _cached_nc[debug]


def run(inputs, debug=False):
    nc = get_nc(debug=debug)
    in_maps, row_sets = shard_inputs(inputs)
    res = run_bass_kernel_spmd(nc, in_maps, core_ids=list(range(8)))
    x = np.asarray(inputs["x"], np.float32)
    out = np.empty_like(x)
    for c in range(8):
        b, rows = row_sets[c]
        out[b][rows] = res.results[c]["outT"].T
    return out, res, row_sets


def kernel(**inputs):
    out, _, _ = run(inputs, debug=False)
    return out


# revision 4
# speedup vs baseline: 1.0567x; 1.0430x over previous
"""Trainium2 kernel v2 for the dense transformer block (B=4, T=2048, C=1024,
H=16, MLP 4x, hybrid cond/causal mask), SPMD over 8 cores, collective-free.

Core (b, rho) = (core//2, core%2) handles batch b, query chunks {2*qi+rho}.
The T axis is chunk-permuted per core (pair-swap for rho=0) so ONE program
(query chunk at position 2*qi+1, kv limit 2*qi+2) serves both rho values.
All GEMMs run in fp8e4m3 DoubleRow (weights scaled 2^12 host-side); masks are
added as -240 bias via PE identity-matmuls before the softmax exp.
"""
import sys
sys.path.insert(0, '/opt/trn_rl_repo')
import numpy as np
import ml_dtypes
import concourse.bass as bass
import concourse.mybir as mybir
import concourse.tile as tile
from concourse.vector_clock import ScopedClock
from concourse.bass_utils import run_bass_kernel_spmd

# ---- walrus 1-sync-wait-per-instruction workarounds (from baseline) ----
_installed = False


def _split_multi_waits(ordered_by_block, nc):
    for bb_name, insts in ordered_by_block.items():
        need = any(
            inst.sync_info is not None and len(inst.sync_info.on_wait) > 1
            for inst in insts
        )
        if not need:
            continue
        new_list = []
        for inst in insts:
            si = inst.sync_info
            waits = list(si.on_wait) if si is not None and si.on_wait else []
            if len(waits) > 1:
                for w in waits[:-1]:
                    nop = mybir.InstNoOp(
                        name=nc.get_next_instruction_name(),
                        ins=[],
                        outs=[],
                        bass_nofuse=True,
                    )
                    nop.engine = inst.engine
                    nop.sync_info = mybir.SyncInfo(on_wait=[w], on_update=[])
                    new_list.append(nop)
                ups = list(si.on_update) if si.on_update else []
                inst.sync_info = mybir.SyncInfo(on_wait=[waits[-1]], on_update=ups)
            new_list.append(inst)
        insts[:] = new_list


class _SplitWaitClockWait:
    def __init__(self, tc, ordered, **kw):
        import bass_rust
        self._inner = bass_rust.TileClockWait(tc, ordered, **kw)
        self._tc = tc
        self._ordered = ordered

    def __getattr__(self, a):
        return getattr(self._inner, a)

    def assign_waits(self, bb_name):
        r = self._inner.assign_waits(bb_name)
        _split_multi_waits(self._ordered, self._tc.nc)
        return r


class PatchedTileContext(tile.TileContext):
    """TileContext whose final drain carries at most one sem wait."""

    def _drain_and_barrier(self, tick_clock, wait_clock):
        probe = self.nc.sync.nop(nofuse=True)
        add = wait_clock.add_sem_waits
        add(probe.ins, ScopedClock({None: tick_clock.global_clock}))
        si = probe.ins.sync_info
        waits = list(si.on_wait) if si is not None and si.on_wait else []
        if len(waits) > 1:
            probe.ins.sync_info = mybir.SyncInfo(on_wait=[waits[0]], on_update=[])
            for w in waits[1:]:
                n = self.nc.sync.nop(nofuse=True)
                n.ins.sync_info = mybir.SyncInfo(on_wait=[w], on_update=[])
        self.nc.sync.drain()
        self.nc.all_engine_barrier()
        popped = self.nc._tile_sem_poison_stack.pop()
        assert popped is self._sem_poison
        self.nc.clear_and_free_semaphores(list(self.sems.allocated().values()))
        self.nc.all_engine_barrier()


def _install():
    global _installed
    if not _installed:
        tile.TileClockWait = _SplitWaitClockWait
        _installed = True


_install()

# ---- constants ----

F32 = mybir.dt.float32
BF16 = mybir.dt.bfloat16
FP8 = mybir.dt.float8e4
E4 = ml_dtypes.float8_e4m3
AF = mybir.ActivationFunctionType
ALU = mybir.AluOpType
DRM = mybir.MatmulPerfMode.DoubleRow

C = 1024
T = 2048
H = 16
HD = 64
FF = 4096
COND = 256
EPS = 1e-5
SCALE = 1.0 / np.sqrt(HD)
WS = 2.0 ** 12       # weight scale
WSI = float(2.0 ** -12)
MNEG = -240.0
NQI = 8              # query chunks per core
# cvec column indices in the packed [128, 88] bias tensor
CV = {"bq": 0, "bp": 8, "b2": 16, "g1": 24, "bt1": 32, "g2": 40, "bt2": 48, "b1": 56}


def build_nc(debug=False):
    nc = bass.Bass()
    xT = nc.dram_tensor("xT", [C, T], F32, kind="ExternalInput")
    wq = nc.dram_tensor("wq", [128, 8, 4, 2, 128], FP8, kind="ExternalInput")
    wk = nc.dram_tensor("wk", [128, 8, 4, 2, 128], FP8, kind="ExternalInput")
    wv = nc.dram_tensor("wv", [128, 4, 4, 2, 256], FP8, kind="ExternalInput")
    wp = nc.dram_tensor("wp", [128, 8, 4, 2, 128], FP8, kind="ExternalInput")
    w1 = nc.dram_tensor("w1", [128, 32, 4, 2, 128], FP8, kind="ExternalInput")
    w2 = nc.dram_tensor("w2", [128, 8, 16, 2, 128], FP8, kind="ExternalInput")
    cvec = nc.dram_tensor("cvec", [128, 88], F32, kind="ExternalInput")
    bvb = nc.dram_tensor("bvb", [128, C], BF16, kind="ExternalInput")
    mneg = nc.dram_tensor("mneg", [14 * 128, 128], FP8, kind="ExternalInput")
    i128 = nc.dram_tensor("i128", [128, 128], FP8, kind="ExternalInput")
    outT = nc.dram_tensor("outT", [C, 1024], F32, kind="ExternalOutput")

    stats_d = nc.dram_tensor("stats_d", [2, T + 1024], BF16)   # a=rstd rows, b=-mu*rstd

    dbg = {}
    if debug:
        dbg["hT"] = nc.dram_tensor("dbg_hT", [C, T], F32, kind="ExternalOutput")
        dbg["v"] = nc.dram_tensor("dbg_v", [T, H * HD], F32, kind="ExternalOutput")
        dbg["yT"] = nc.dram_tensor("dbg_yT", [C, 1024], F32, kind="ExternalOutput")
        dbg["x2T"] = nc.dram_tensor("dbg_x2T", [C, 1024], F32, kind="ExternalOutput")

    with PatchedTileContext(nc) as tc:
        import contextlib
        with contextlib.ExitStack() as _es:
            _e = _es.enter_context
            _e(nc.allow_low_precision(reason="bf16/fp8 intermediates validated vs numpy"))
            big = _e(tc.tile_pool(name="big", bufs=1))
            wbig = _e(tc.tile_pool(name="wbig", bufs=1))
            w1s_p = _e(tc.tile_pool(name="w1s", bufs=2))
            w2s_p = _e(tc.tile_pool(name="w2s", bufs=2))
            st = _e(tc.tile_pool(name="st", bufs=1))
            xin_p = _e(tc.tile_pool(name="xin", bufs=2))
            xb_p = _e(tc.tile_pool(name="xb", bufs=2))
            bc_p = _e(tc.tile_pool(name="bc", bufs=2))
            rows_p = _e(tc.tile_pool(name="rows", bufs=1))
            se_p = _e(tc.tile_pool(name="se", bufs=2))
            ktmp_p = _e(tc.tile_pool(name="ktmp", bufs=2))
            sml_p = _e(tc.tile_pool(name="sml", bufs=2))
            out_p = _e(tc.tile_pool(name="outp", bufs=2))
            pss = _e(tc.tile_pool(name="pss", bufs=2, space="PSUM"))
            psy = _e(tc.tile_pool(name="psy", bufs=1, space="PSUM"))
            ps1 = _e(tc.tile_pool(name="ps1", bufs=2, space="PSUM"))
            # ---------------- resident tensors ----------------
            hT = big.tile([128, 8, T], FP8, tag="A")
            v_ext = big.tile([128, 16, 16, 80], FP8, tag="V")
            kst = [big.tile([128, 2, T], FP8, tag=f"K{t4}", name=f"kst{t4}") for t4 in range(4)]
            qst = [big.tile([128, 2, NQI, 128], FP8, tag=f"Q{t4}", name=f"qst{t4}") for t4 in range(4)]
            yT = big.tile([128, 8, 1024], FP8, tag="Y")
            x2 = big.tile([128, 8, 1024], BF16, tag="X2")
            h2T = big.tile([128, 8, 1024], FP8, tag="H2")
            wq_sb = wbig.tile([128, 8, 4, 2, 128], FP8, tag="WQ")
            wk_sb = wbig.tile([128, 8, 4, 2, 128], FP8, tag="WK")
            wv_sb = wbig.tile([128, 4, 4, 2, 256], FP8, tag="WV")
            wp_sb = wbig.tile([128, 8, 4, 2, 128], FP8, tag="WP")
            cv = st.tile([128, 88], F32)
            bvb_sb = st.tile([128, C], BF16)
            mneg_sb = st.tile([128, 14, 128], FP8)
            i128_sb = st.tile([128, 128], FP8)
            ones1 = st.tile([128, 1], BF16)
            ones64 = st.tile([1, 64], BF16)
            eps1 = st.tile([1, 1], F32)

            nc.vector.memset(ones1, 1.0)
            nc.vector.memset(ones64, 1.0)
            nc.vector.memset(eps1, EPS)
            nc.sync.dma_start(out=wq_sb, in_=wq[:, :, :, :, :])
            nc.sync.dma_start(out=wk_sb, in_=wk[:, :, :, :, :])
            nc.sync.dma_start(out=wv_sb, in_=wv[:, :, :, :, :])
            nc.sync.dma_start(out=wp_sb, in_=wp[:, :, :, :, :])
            nc.sync.dma_start(out=cv, in_=cvec[:, :])
            nc.sync.dma_start(out=bvb_sb, in_=bvb[:, :])
            nc.sync.dma_start(out=mneg_sb, in_=mneg.rearrange("(s p) k -> p s k", p=128))
            nc.sync.dma_start(out=i128_sb, in_=i128[:, :])

            def cvap(name, i):
                return cv[:, CV[name] + i:CV[name] + i + 1]

            # ---------------- layernorm (x source: dram f32 or sbuf bf16) ------
            def ln(src_dram, src_sb, col0, ncols, stat_off, out_tile, gname, btname,
                   after_tile=None, sq_on_act=False):
                """src [C, *]; normalizes cols [col0, col0+ncols) writing fp8
                (affine g/bt) into out_tile[:, cc, same cols]. Stats broadcast
                via stats_d rows at stat_off+col."""
                nt = ncols // 512
                for t in range(nt):
                    cs = slice(col0 + 512 * t, col0 + 512 * (t + 1))
                    stat_ps = pss.tile([33, 512], F32, tag="sps")
                    mu_ps = stat_ps[0:1, :]
                    sq_ps = stat_ps[32:33, :]
                    xbt = xb_p.tile([128, 8, 512], BF16, tag="xb")
                    for cc in range(8):
                        if src_dram is not None:
                            xt = xin_p.tile([128, 512], F32, tag="x")
                            nc.sync.dma_start(
                                out=xt, in_=src_dram[128 * cc:128 * (cc + 1), cs])
                            nc.vector.tensor_copy(out=xbt[:, cc, :], in_=xt)
                        else:
                            nc.vector.tensor_copy(
                                out=xbt[:, cc, :], in_=src_sb[:, cc, cs])
                        sq = xin_p.tile([128, 512], BF16, tag="sq")
                        if sq_on_act:
                            nc.scalar.activation(out=sq, in_=xbt[:, cc, :],
                                                 func=AF.Square)
                        else:
                            nc.vector.tensor_mul(sq, xbt[:, cc, :], xbt[:, cc, :])
                        nc.tensor.matmul(mu_ps, ones1, xbt[:, cc, :],
                                         start=(cc == 0), stop=(cc == 7))
                        nc.tensor.matmul(sq_ps, ones1, sq,
                                         start=(cc == 0), stop=(cc == 7))
                    mu = rows_p.tile([1, 512], F32, tag="mu")
                    ex2 = rows_p.tile([1, 512], F32, tag="t2")
                    nc.vector.tensor_scalar_mul(mu, mu_ps, 1.0 / C)
                    nc.vector.tensor_scalar_mul(ex2, sq_ps, 1.0 / C)
                    var = rows_p.tile([1, 512], F32, tag="t3")
                    nc.vector.scalar_tensor_tensor(
                        out=var, in0=mu, scalar=-1.0, in1=mu, op0=ALU.mult, op1=ALU.mult)
                    nc.vector.tensor_add(var, var, ex2)
                    std = rows_p.tile([1, 512], F32, tag="t2")
                    nc.scalar.activation(out=std, in_=var, func=AF.Sqrt, bias=eps1)
                    arow = rows_p.tile([1, 512], BF16, tag="ab16")
                    nc.vector.reciprocal(out=arow, in_=std)
                    brow = rows_p.tile([1, 512], BF16, tag="ab16b")
                    nc.vector.scalar_tensor_tensor(
                        out=brow, in0=mu, scalar=-1.0, in1=arow,
                        op0=ALU.mult, op1=ALU.mult)
                    so = stat_off + col0 + 512 * t
                    nc.sync.dma_start(out=stats_d[0, so:so + 512][None, :], in_=arow)
                    nc.sync.dma_start(out=stats_d[1, so:so + 512][None, :], in_=brow)
                    a_b = bc_p.tile([128, 512], BF16, tag="a_b")
                    b_b = bc_p.tile([128, 512], BF16, tag="b_b")
                    nc.sync.dma_start(out=a_b, in_=bass.AP(
                        tensor=stats_d[0][None, :].tensor,
                        offset=so, ap=[[0, 128], [1, 512]]))
                    nc.sync.dma_start(out=b_b, in_=bass.AP(
                        tensor=stats_d[1][None, :].tensor,
                        offset=(T + 1024) + so, ap=[[0, 128], [1, 512]]))
                    for cc in range(8):
                        t1 = xin_p.tile([128, 512], BF16, tag="t1")
                        nc.vector.tensor_mul(t1, xbt[:, cc, :], a_b)
                        u = xin_p.tile([128, 512], BF16, tag="u")
                        nc.vector.tensor_add(u, t1, b_b)
                        nc.gpsimd.tensor_scalar(
                            out_tile[:, cc, cs], u, cvap(gname, cc), cvap(btname, cc),
                            op0=ALU.mult, op1=ALU.add)
                    if after_tile is not None:
                        after_tile(t)

            def emit_v(m):
                for fh in range(4):
                    vps = ps1.tile([128, 256], F32, tag="pmid")
                    for g in range(4):
                        nc.tensor.matmul(
                            vps, hT[:, 2 * g:2 * g + 2, 128 * m:128 * (m + 1)],
                            wv_sb[:, fh, g, :, :],
                            start=(g == 0), stop=(g == 3), perf_mode=DRM,
                            skip_group_check=True)
                    nc.vector.scalar_tensor_tensor(
                        out=v_ext[:, m, 4 * fh:4 * fh + 4, 0:HD],
                        in0=vps.rearrange("p (h d) -> p h d", h=4), scalar=WSI,
                        in1=bvb_sb[:, 256 * fh:256 * (fh + 1)].rearrange(
                            "p (h d) -> p h d", h=4),
                        op0=ALU.mult, op1=ALU.add)
                nc.vector.memset(v_ext[:, m, :, HD:HD + 1], 1.0)

            def emit_qh(qh):
                for p in range(8):
                    qtmp = ktmp_p.tile([128, 4, 128], FP8, tag="qt")
                    qps = ps1.tile([128, 4, 128], F32, tag="pmid")
                    for qi4 in range(4):
                        qi = 4 * qh + qi4
                        qc0 = 128 * (2 * qi + 1)
                        for g in range(4):
                            nc.tensor.matmul(
                                qps[:, qi4, :], wq_sb[:, p, g, :, :],
                                hT[:, 2 * g:2 * g + 2, qc0:qc0 + 128],
                                start=(g == 0), stop=(g == 3), perf_mode=DRM,
                                skip_group_check=True)
                    nc.vector.tensor_scalar(
                        qtmp.rearrange("p a b -> p (a b)"),
                        qps.rearrange("p a b -> p (a b)"),
                        WSI, cvap("bq", p), op0=ALU.mult, op1=ALU.add)
                    for odd in range(2):
                        h = 2 * p + odd
                        t4, h4 = h // 4, h % 4
                        for i in range(2):
                            nc.sync.dma_start(
                                out=qst[t4][32 * h4:32 * h4 + 32, i,
                                            4 * qh:4 * qh + 4, :],
                                in_=qtmp[64 * odd + 32 * i:64 * odd + 32 * i + 32,
                                         :, :])

            def emit_k_half(th):
                for p in range(8):
                    ktmp = ktmp_p.tile([128, 1024], FP8, tag="kt")
                    for tt in range(2):
                        t = 2 * th + tt
                        kps = ps1.tile([128, 512], F32, tag="pmid")
                        for half in range(2):
                            cs = slice(512 * t + 256 * half,
                                       512 * t + 256 * (half + 1))
                            for g in range(4):
                                nc.tensor.matmul(
                                    kps[:, 256 * half:256 * (half + 1)],
                                    wk_sb[:, p, g, :, :], hT[:, 2 * g:2 * g + 2, cs],
                                    start=(g == 0), stop=(g == 3), perf_mode=DRM,
                                    skip_group_check=True)
                        nc.vector.tensor_scalar_mul(
                            ktmp[:, 512 * tt:512 * (tt + 1)], kps, WSI)
                    for odd in range(2):
                        h = 2 * p + odd
                        t4, h4 = h // 4, h % 4
                        for i in range(2):
                            nc.sync.dma_start(
                                out=kst[t4][32 * h4:32 * h4 + 32, i,
                                            1024 * th:1024 * (th + 1)],
                                in_=ktmp[64 * odd + 32 * i:64 * odd + 32 * i + 32, :])

            def emit_k_quarter(t):
                for p in range(8):
                    ktq = ktmp_p.tile([128, 512], FP8, tag="kt")
                    kps = ps1.tile([128, 512], F32, tag="pmid")
                    for half in range(2):
                        cs = slice(512 * t + 256 * half, 512 * t + 256 * (half + 1))
                        for g in range(4):
                            nc.tensor.matmul(
                                kps[:, 256 * half:256 * (half + 1)],
                                wk_sb[:, p, g, :, :], hT[:, 2 * g:2 * g + 2, cs],
                                start=(g == 0), stop=(g == 3), perf_mode=DRM,
                                skip_group_check=True)
                    nc.vector.tensor_scalar_mul(ktq, kps, WSI)
                    for odd in range(2):
                        h = 2 * p + odd
                        t4, h4 = h // 4, h % 4
                        for i in range(2):
                            nc.sync.dma_start(
                                out=kst[t4][32 * h4:32 * h4 + 32, i,
                                            512 * t:512 * (t + 1)],
                                in_=ktq[64 * odd + 32 * i:64 * odd + 32 * i + 32, :])

            def emit_qp(qi0):
                for p in range(8):
                    qtmp = ktmp_p.tile([128, 2, 128], FP8, tag="qt")
                    qps = ps1.tile([128, 2, 128], F32, tag="pmid")
                    for qi4 in range(2):
                        qi = qi0 + qi4
                        qc0 = 128 * (2 * qi + 1)
                        for g in range(4):
                            nc.tensor.matmul(
                                qps[:, qi4, :], wq_sb[:, p, g, :, :],
                                hT[:, 2 * g:2 * g + 2, qc0:qc0 + 128],
                                start=(g == 0), stop=(g == 3), perf_mode=DRM,
                                skip_group_check=True)
                    nc.vector.tensor_scalar(
                        qtmp.rearrange("p a b -> p (a b)"),
                        qps.rearrange("p a b -> p (a b)"),
                        WSI, cvap("bq", p), op0=ALU.mult, op1=ALU.add)
                    for odd in range(2):
                        h = 2 * p + odd
                        t4, h4 = h // 4, h % 4
                        for i in range(2):
                            nc.sync.dma_start(
                                out=qst[t4][32 * h4:32 * h4 + 32, i,
                                            qi0:qi0 + 2, :],
                                in_=qtmp[64 * odd + 32 * i:64 * odd + 32 * i + 32,
                                         :, :])

            def ln1_after(t):
                for m in range(4 * t, 4 * t + 4):
                    emit_v(m)
                if t == 1:
                    emit_k_half(0)
                    emit_qh(0)
                if t == 3:
                    emit_k_half(1)
                    emit_qh(1)

            ln(xT, None, 0, T, 0, hT, "g1", "bt1", after_tile=ln1_after, sq_on_act=True)

            if debug:
                for cc in range(8):
                    for t in range(4):
                        dt_ = out_p.tile([128, 512], F32, tag="dbg")
                        nc.vector.tensor_copy(out=dt_, in_=hT[:, cc, 512 * t:512 * (t + 1)])
                        nc.sync.dma_start(
                            out=dbg["hT"][128 * cc:128 * (cc + 1), 512 * t:512 * (t + 1)],
                            in_=dt_)


            if debug:
                for m in range(16):
                    dt_ = out_p.tile([128, H * HD], F32, tag="dbgv")
                    nc.vector.tensor_copy(
                        out=dt_.rearrange("p (h d) -> p h d", h=H),
                        in_=v_ext[:, m, :, 0:HD])
                    nc.sync.dma_start(out=dbg["v"][128 * m:128 * (m + 1), :], in_=dt_)

            # ---------------- attention + proj + LN2 + MLP, pipelined by qi ----
            def attn_qi(qi):
                L = 2 * qi + 2
                for hhalf in range(2):
                    yall = psy.tile([65, 8, 128], F32, tag="yall")
                    for hh in range(8):
                        h = 8 * hhalf + hh
                        t4, h4 = h // 4, h % 4
                        se = se_p.tile([128, 16, 128], FP8, tag="se")
                        for off in range(0, L, 8):
                            pl = min(8, L - off)
                            sps = pss.tile([128, 8, 128], F32, tag="sps")
                            for mi in range(pl):
                                m = off + mi
                                masked = qi >= 1 and m >= 2 * qi
                                nc.tensor.matmul(
                                    sps[:, mi, :],
                                    kst[t4][32 * h4:32 * h4 + 32, :,
                                            128 * m:128 * (m + 1)],
                                    qst[t4][32 * h4:32 * h4 + 32, :, qi, :],
                                    start=True, stop=not masked, perf_mode=DRM,
                                    tile_position=(32 * h4, 0),
                                    skip_group_check=True)
                                if masked:
                                    slot = 2 * (qi - 1) + (m - 2 * qi)
                                    nc.tensor.matmul(
                                        sps[:, mi, :], mneg_sb[:, slot, :], i128_sb,
                                        start=False, stop=True,
                                        skip_group_check=True)
                            nc.scalar.activation(
                                out=se[:, off:off + pl, :], in_=sps[:, 0:pl, :],
                                func=AF.Exp, scale=float(SCALE))
                        for g in range(L // 2):
                            nc.tensor.matmul(
                                yall[0:65, hh, :],
                                v_ext[:, 2 * g:2 * g + 2, h, 0:65],
                                se[:, 2 * g:2 * g + 2, :],
                                start=(g == 0 and hh % 4 == 0),
                                stop=(g == L // 2 - 1 and hh % 4 == 3),
                                perf_mode=DRM, skip_group_check=True)
                    # normalize 8 heads at once; free yall (PSUM) early by
                    # staging sums+values to SBUF, broadcast recips via PE
                    rs = sml_p.tile([1, 1024], BF16, tag="rs")
                    nc.vector.reciprocal(
                        out=rs.rearrange("p (o a b) -> p o a b", o=2, a=4),
                        in_=yall[64:65, :, :].rearrange("p (a o) b -> p o a b",
                                                        o=2))
                    ysb = sml_p.tile([64, 8, 128], BF16, tag="ysb")
                    nc.vector.tensor_copy(out=ysb, in_=yall[0:64, :, :])
                    yv = ysb.rearrange("p (a o) b -> p a o b", o=2)
                    for o in range(2):
                        rbps = ps1.tile([64, 512], F32, tag="pmid")
                        nc.tensor.matmul(rbps, ones64, rs[:, 512 * o:512 * (o + 1)],
                                         start=True, stop=True)
                        rv = rbps.rearrange("p (a b) -> p a b", a=4)
                        if o == 0:
                            nc.vector.tensor_mul(
                                yT[0:64, 4 * hhalf:4 * hhalf + 4,
                                   128 * qi:128 * (qi + 1)],
                                yv[:, :, 0, :], rv)
                        else:
                            yo = sml_p.tile([64, 4, 128], FP8, tag="yo")
                            nc.vector.tensor_mul(yo, yv[:, :, 1, :], rv)
                            nc.sync.dma_start(
                                out=yT[64:128, 4 * hhalf:4 * hhalf + 4,
                                       128 * qi:128 * (qi + 1)],
                                in_=yo)

            def proj_t(t):
                for co in range(8):
                    pps = ps1.tile([128, 512], F32, tag="pmid")
                    for half in range(2):
                        qs = slice(512 * t + 256 * half, 512 * t + 256 * (half + 1))
                        for g in range(4):
                            nc.tensor.matmul(
                                pps[:, 256 * half:256 * (half + 1)],
                                wp_sb[:, co, g, :, :], yT[:, 2 * g:2 * g + 2, qs],
                                start=(g == 0), stop=(g == 3), perf_mode=DRM,
                                skip_group_check=True)
                    xq = xin_p.tile([128, 512], F32, tag="xq")
                    nc.sync.dma_start(out=xq, in_=bass.AP(
                        tensor=xT[:, :].tensor,
                        offset=(128 * co) * T + 128 * (8 * t + 1),
                        ap=[[T, 128], [256, 4], [1, 128]]))
                    s_ = xin_p.tile([128, 512], F32, tag="s_")
                    nc.vector.scalar_tensor_tensor(
                        out=s_, in0=pps, scalar=WSI, in1=xq,
                        op0=ALU.mult, op1=ALU.add)
                    nc.gpsimd.tensor_scalar_add(
                        x2[:, co, 512 * t:512 * (t + 1)], s_, cvap("bp", co))
                    if debug:
                        dt_ = out_p.tile([128, 512], F32, tag="dbg")
                        nc.vector.tensor_scalar_add(dt_, s_, cvap("bp", co))
                        nc.sync.dma_start(
                            out=dbg["x2T"][128 * co:128 * (co + 1),
                                           512 * t:512 * (t + 1)],
                            in_=dt_)

            def fc1_t(t):
                gA = big.tile([128, 32, 512], FP8, tag="A")  # reuses hT slot
                for f in range(32):
                    w1_sl = w1s_p.tile([128, 4, 2, 128], FP8, tag="w1")
                    nc.sync.dma_start(out=w1_sl, in_=w1[:, f, :, :, :])
                    fps = ps1.tile([128, 512], F32, tag="pmid")
                    for half in range(2):
                        qs = slice(512 * t + 256 * half, 512 * t + 256 * (half + 1))
                        for g in range(4):
                            nc.tensor.matmul(
                                fps[:, 256 * half:256 * (half + 1)],
                                w1_sl[:, g, :, :], h2T[:, 2 * g:2 * g + 2, qs],
                                start=(g == 0), stop=(g == 3), perf_mode=DRM,
                                skip_group_check=True)
                    nc.scalar.activation(
                        out=gA[:, f, :], in_=fps, func=AF.Gelu,
                        bias=cvap("b1", f), scale=WSI)
                return gA

            def fc2_t(t, gA):
                for co in range(8):
                    ops_ = ps1.tile([128, 512], F32, tag="pmid")
                    w2_sl = w2s_p.tile([128, 16, 2, 128], FP8, tag="w2")
                    nc.sync.dma_start(out=w2_sl, in_=w2[:, co, :, :, :])
                    for half in range(2):
                        for g in range(16):
                            nc.tensor.matmul(
                                ops_[:, 256 * half:256 * (half + 1)],
                                w2_sl[:, g, :, :],
                                gA[:, 2 * g:2 * g + 2,
                                   256 * half:256 * (half + 1)],
                                start=(g == 0), stop=(g == 15), perf_mode=DRM,
                                skip_group_check=True)
                    o1 = xin_p.tile([128, 512], F32, tag="s_")
                    nc.vector.scalar_tensor_tensor(
                        out=o1, in0=ops_, scalar=WSI,
                        in1=x2[:, co, 512 * t:512 * (t + 1)],
                        op0=ALU.mult, op1=ALU.add)
                    o2 = out_p.tile([128, 512], F32, tag="o2")
                    nc.gpsimd.tensor_scalar_add(o2, o1, cvap("b2", co))
                    nc.sync.dma_start(
                        out=outT[128 * co:128 * (co + 1), 512 * t:512 * (t + 1)],
                        in_=o2)

            for qi in range(4):
                attn_qi(qi)
            attn_qi(4)
            proj_t(0)
            attn_qi(5)
            ln(None, x2, 0, 512, T, h2T, "g2", "bt2")
            attn_qi(6)
            gA0 = fc1_t(0)
            attn_qi(7)
            proj_t(1)
            ln(None, x2, 512, 512, T, h2T, "g2", "bt2")
            fc2_t(0, gA0)

            if debug:
                for p in range(8):
                    for t in range(2):
                        dt_ = out_p.tile([128, 512], F32, tag="dbg")
                        nc.vector.tensor_copy(
                            out=dt_, in_=yT[:, p, 512 * t:512 * (t + 1)])
                        nc.sync.dma_start(
                            out=dbg["yT"][128 * p:128 * (p + 1),
                                          512 * t:512 * (t + 1)],
                            in_=dt_)

            gA1 = fc1_t(1)
            fc2_t(1, gA1)

    return nc


# ===================== host side =====================

def make_mneg(rho, cond):
    """14 slots [(qi,ms) for qi in 1..7 for ms in 0,1], each stored transposed
    [q, k] fp8: 0 where allowed, -240 where masked. Key chunk at permuted
    position m corresponds to natural chunk (pairswap(m) if rho==0 else m)."""
    out = np.zeros((14, 128, 128), dtype=np.float32)
    for qi in range(1, 8):
        qnat = 2 * qi + rho          # natural q chunk
        qrows = 128 * qnat + np.arange(128)
        for ms in range(2):
            m = 2 * qi + ms          # permuted key-chunk position
            n = (m ^ 1) if rho == 0 else m   # natural key chunk
            krows = 128 * n + np.arange(128)
            i = qrows[None, :]       # [1, q] -> transposed layout [q, k] idx [k, q]?
            # allowed(i=query, j=key); stored slot[q, k]
            qq = qrows[:, None]
            kk = krows[None, :]
            allowed = (kk < cond) | ((qq >= cond) & (kk >= cond) & (kk <= qq))
            out[2 * (qi - 1) + ms] = np.where(allowed, 0.0, MNEG)
    return out.reshape(14 * 128, 128).astype(E4)


def pack_dr_w(W, gdim, fblk):
    """W [1024 or 4096, F] -> [128, nf, g, 2, fblk] with c = 256g+128i+p."""
    Cin, F = W.shape
    ng = Cin // 256
    nf = F // fblk
    out = np.empty((128, nf, ng, 2, fblk), dtype=E4)
    Ws = (W * WS).astype(E4)
    for fi in range(nf):
        blk = Ws[:, fblk * fi:fblk * (fi + 1)]          # [Cin, fblk]
        r = blk.reshape(ng, 2, 128, fblk)                # g, i, p, f
        out[:, fi] = r.transpose(2, 0, 1, 3)             # p, g, i, f
    return np.ascontiguousarray(out)


def perm_cols(rho):
    """column permutation of T at 128 granularity: rho=0 pairswap, rho=1 id."""
    chunks = np.arange(16)
    if rho == 0:
        chunks = chunks ^ 1
    idx = (128 * chunks[:, None] + np.arange(128)[None, :]).reshape(-1)
    return idx


def shard_inputs(inputs):
    x = np.asarray(inputs["x"], np.float32)
    cond = int(np.asarray(inputs["cond_len"]))
    cvec = np.zeros((128, 88), np.float32)
    for name, key in [("bq", "bq"), ("bp", "bp"), ("b2", "b2"), ("g1", "ln1_g"),
                      ("bt1", "ln1_b"), ("g2", "ln2_g"), ("bt2", "ln2_b")]:
        v = np.asarray(inputs[key], np.float32)
        cvec[:, CV[name]:CV[name] + 8] = v.reshape(8, 128).T
    cvec[:, CV["b1"]:CV["b1"] + 32] = np.asarray(
        inputs["b1"], np.float32).reshape(32, 128).T
    common = {
        "wq": pack_dr_w(np.asarray(inputs["Wq"], np.float32), 4, 128),
        "wk": pack_dr_w(np.asarray(inputs["Wk"], np.float32), 4, 128),
        "wv": pack_dr_w(np.asarray(inputs["Wv"], np.float32), 4, 256),
        "wp": pack_dr_w(np.asarray(inputs["Wp"], np.float32), 4, 128),
        "w1": pack_dr_w(np.asarray(inputs["W1"], np.float32), 4, 128),
        "w2": pack_dr_w(np.asarray(inputs["W2"], np.float32), 16, 128),
        "cvec": cvec,
        "bvb": np.ascontiguousarray(np.broadcast_to(
            np.asarray(inputs["bv"], np.float32), (128, C))).astype(
                ml_dtypes.bfloat16),
        "i128": np.eye(128, dtype=np.float32).astype(E4),
    }
    in_maps, row_sets = [], []
    for c in range(8):
        b, rho = c // 2, c % 2
        m = dict(common)
        cols = perm_cols(rho)
        m["xT"] = np.ascontiguousarray(x[b].T[:, cols])
        m["mneg"] = make_mneg(rho, cond)
        rows = np.concatenate(
            [np.arange(128 * (2 * qi + rho), 128 * (2 * qi + rho) + 128)
             for qi in range(8)])
        row_sets.append((b, rows))
        in_maps.append(m)
    return in_maps, row_sets


_cached_nc = {}


def get_nc(debug=False):
    if debug not in _cached_nc:
        _cached_nc[debug] = build_nc(debug=debug)
    return _cached_nc[debug]


def run(inputs, debug=False):
    nc = get_nc(debug=debug)
    in_maps, row_sets = shard_inputs(inputs)
    res = run_bass_kernel_spmd(nc, in_maps, core_ids=list(range(8)))
    x = np.asarray(inputs["x"], np.float32)
    out = np.empty_like(x)
    for c in range(8):
        b, rows = row_sets[c]
        out[b][rows] = res.results[c]["outT"].T
    return out, res, row_sets


def kernel(**inputs):
    out, _, _ = run(inputs, debug=False)
    return out


# revision 5
# speedup vs baseline: 1.0691x; 1.0117x over previous
"""Trainium2 kernel v2 for the dense transformer block (B=4, T=2048, C=1024,
H=16, MLP 4x, hybrid cond/causal mask), SPMD over 8 cores, collective-free.

Core (b, rho) = (core//2, core%2) handles batch b, query chunks {2*qi+rho}.
The T axis is chunk-permuted per core (pair-swap for rho=0) so ONE program
(query chunk at position 2*qi+1, kv limit 2*qi+2) serves both rho values.
All GEMMs run in fp8e4m3 DoubleRow (weights scaled 2^12 host-side); masks are
added as -240 bias via PE identity-matmuls before the softmax exp.
"""
import sys
sys.path.insert(0, '/opt/trn_rl_repo')
import numpy as np
import ml_dtypes
import concourse.bass as bass
import concourse.mybir as mybir
import concourse.tile as tile
from concourse.vector_clock import ScopedClock
from concourse.bass_utils import run_bass_kernel_spmd

# ---- walrus 1-sync-wait-per-instruction workarounds (from baseline) ----
_installed = False


def _split_multi_waits(ordered_by_block, nc):
    for bb_name, insts in ordered_by_block.items():
        need = any(
            inst.sync_info is not None and len(inst.sync_info.on_wait) > 1
            for inst in insts
        )
        if not need:
            continue
        new_list = []
        for inst in insts:
            si = inst.sync_info
            waits = list(si.on_wait) if si is not None and si.on_wait else []
            if len(waits) > 1:
                for w in waits[:-1]:
                    nop = mybir.InstNoOp(
                        name=nc.get_next_instruction_name(),
                        ins=[],
                        outs=[],
                        bass_nofuse=True,
                    )
                    nop.engine = inst.engine
                    nop.sync_info = mybir.SyncInfo(on_wait=[w], on_update=[])
                    new_list.append(nop)
                ups = list(si.on_update) if si.on_update else []
                inst.sync_info = mybir.SyncInfo(on_wait=[waits[-1]], on_update=ups)
            new_list.append(inst)
        insts[:] = new_list


class _SplitWaitClockWait:
    def __init__(self, tc, ordered, **kw):
        import bass_rust
        self._inner = bass_rust.TileClockWait(tc, ordered, **kw)
        self._tc = tc
        self._ordered = ordered

    def __getattr__(self, a):
        return getattr(self._inner, a)

    def assign_waits(self, bb_name):
        r = self._inner.assign_waits(bb_name)
        _split_multi_waits(self._ordered, self._tc.nc)
        return r


class PatchedTileContext(tile.TileContext):
    """TileContext whose final drain carries at most one sem wait."""

    def _drain_and_barrier(self, tick_clock, wait_clock):
        probe = self.nc.sync.nop(nofuse=True)
        add = wait_clock.add_sem_waits
        add(probe.ins, ScopedClock({None: tick_clock.global_clock}))
        si = probe.ins.sync_info
        waits = list(si.on_wait) if si is not None and si.on_wait else []
        if len(waits) > 1:
            probe.ins.sync_info = mybir.SyncInfo(on_wait=[waits[0]], on_update=[])
            for w in waits[1:]:
                n = self.nc.sync.nop(nofuse=True)
                n.ins.sync_info = mybir.SyncInfo(on_wait=[w], on_update=[])
        self.nc.sync.drain()
        self.nc.all_engine_barrier()
        popped = self.nc._tile_sem_poison_stack.pop()
        assert popped is self._sem_poison
        self.nc.clear_and_free_semaphores(list(self.sems.allocated().values()))
        self.nc.all_engine_barrier()


def _install():
    global _installed
    if not _installed:
        tile.TileClockWait = _SplitWaitClockWait
        _installed = True


_install()

# ---- constants ----

F32 = mybir.dt.float32
BF16 = mybir.dt.bfloat16
FP8 = mybir.dt.float8e4
E4 = ml_dtypes.float8_e4m3
AF = mybir.ActivationFunctionType
ALU = mybir.AluOpType
DRM = mybir.MatmulPerfMode.DoubleRow

C = 1024
T = 2048
H = 16
HD = 64
FF = 4096
COND = 256
EPS = 1e-5
SCALE = 1.0 / np.sqrt(HD)
WS = 2.0 ** 12       # weight scale
WSI = float(2.0 ** -12)
MNEG = -240.0
NQI = 8              # query chunks per core
# cvec column indices in the packed [128, 88] bias tensor
CV = {"bq": 0, "bp": 8, "b2": 16, "g1": 24, "bt1": 32, "g2": 40, "bt2": 48, "b1": 56}


def build_nc(debug=False):
    nc = bass.Bass()
    xT = nc.dram_tensor("xT", [C, T], F32, kind="ExternalInput")
    wq = nc.dram_tensor("wq", [128, 8, 4, 2, 128], FP8, kind="ExternalInput")
    wk = nc.dram_tensor("wk", [128, 8, 4, 2, 128], FP8, kind="ExternalInput")
    wv = nc.dram_tensor("wv", [128, 4, 4, 2, 256], FP8, kind="ExternalInput")
    wp = nc.dram_tensor("wp", [128, 8, 4, 2, 128], FP8, kind="ExternalInput")
    w1 = nc.dram_tensor("w1", [128, 32, 4, 2, 128], FP8, kind="ExternalInput")
    w2 = nc.dram_tensor("w2", [128, 8, 16, 2, 128], FP8, kind="ExternalInput")
    cvec = nc.dram_tensor("cvec", [128, 88], F32, kind="ExternalInput")
    bvb = nc.dram_tensor("bvb", [128, C], BF16, kind="ExternalInput")
    mneg = nc.dram_tensor("mneg", [14 * 128, 128], FP8, kind="ExternalInput")
    i128 = nc.dram_tensor("i128", [128, 128], FP8, kind="ExternalInput")
    outT = nc.dram_tensor("outT", [C, 1024], F32, kind="ExternalOutput")

    stats_d = nc.dram_tensor("stats_d", [2, T + 1024], BF16)   # a=rstd rows, b=-mu*rstd

    dbg = {}
    if debug:
        dbg["hT"] = nc.dram_tensor("dbg_hT", [C, T], F32, kind="ExternalOutput")
        dbg["v"] = nc.dram_tensor("dbg_v", [T, H * HD], F32, kind="ExternalOutput")
        dbg["yT"] = nc.dram_tensor("dbg_yT", [C, 1024], F32, kind="ExternalOutput")
        dbg["x2T"] = nc.dram_tensor("dbg_x2T", [C, 1024], F32, kind="ExternalOutput")

    with PatchedTileContext(nc) as tc:
        import contextlib
        with contextlib.ExitStack() as _es:
            _e = _es.enter_context
            _e(nc.allow_low_precision(reason="bf16/fp8 intermediates validated vs numpy"))
            big = _e(tc.tile_pool(name="big", bufs=1))
            wbig = _e(tc.tile_pool(name="wbig", bufs=1))
            w1s_p = _e(tc.tile_pool(name="w1s", bufs=2))
            w2s_p = _e(tc.tile_pool(name="w2s", bufs=2))
            st = _e(tc.tile_pool(name="st", bufs=1))
            xin_p = _e(tc.tile_pool(name="xin", bufs=2))
            xb_p = _e(tc.tile_pool(name="xb", bufs=2))
            bc_p = _e(tc.tile_pool(name="bc", bufs=2))
            rows_p = _e(tc.tile_pool(name="rows", bufs=1))
            se_p = _e(tc.tile_pool(name="se", bufs=2))
            ktmp_p = _e(tc.tile_pool(name="ktmp", bufs=2))
            sml_p = _e(tc.tile_pool(name="sml", bufs=2))
            out_p = _e(tc.tile_pool(name="outp", bufs=2))
            pss = _e(tc.tile_pool(name="pss", bufs=2, space="PSUM"))
            psy = _e(tc.tile_pool(name="psy", bufs=1, space="PSUM"))
            ps1 = _e(tc.tile_pool(name="ps1", bufs=2, space="PSUM"))
            # ---------------- resident tensors ----------------
            hT = big.tile([128, 8, T], FP8, tag="A")
            v_ext = big.tile([128, 16, 16, 80], FP8, tag="V")
            kst = [big.tile([128, 2, T], FP8, tag=f"K{t4}", name=f"kst{t4}") for t4 in range(4)]
            qst = [big.tile([128, 2, NQI, 128], FP8, tag=f"Q{t4}", name=f"qst{t4}") for t4 in range(4)]
            yT = big.tile([128, 8, 1024], FP8, tag="Y")
            x2 = big.tile([128, 8, 1024], BF16, tag="X2")
            h2T = big.tile([128, 8, 1024], FP8, tag="H2")
            wq_sb = wbig.tile([128, 8, 4, 2, 128], FP8, tag="WQ")
            wk_sb = wbig.tile([128, 8, 4, 2, 128], FP8, tag="WK")
            wv_sb = wbig.tile([128, 4, 4, 2, 256], FP8, tag="WV")
            wp_sb = wbig.tile([128, 8, 4, 2, 128], FP8, tag="WP")
            cv = st.tile([128, 88], F32)
            bvb_sb = st.tile([128, C], BF16)
            mneg_sb = st.tile([128, 14, 128], FP8)
            i128_sb = st.tile([128, 128], FP8)
            ones1 = st.tile([128, 1], BF16)
            ones64 = st.tile([1, 64], BF16)
            eps1 = st.tile([1, 1], F32)

            nc.vector.memset(ones1, 1.0)
            nc.vector.memset(ones64, 1.0)
            nc.vector.memset(eps1, EPS)
            nc.scalar.dma_start(out=wq_sb, in_=wq[:, :, :, :, :])
            nc.scalar.dma_start(out=wk_sb, in_=wk[:, :, :, :, :])
            nc.scalar.dma_start(out=wv_sb, in_=wv[:, :, :, :, :])
            nc.scalar.dma_start(out=wp_sb, in_=wp[:, :, :, :, :])
            nc.scalar.dma_start(out=cv, in_=cvec[:, :])
            nc.scalar.dma_start(out=bvb_sb, in_=bvb[:, :])
            nc.scalar.dma_start(out=mneg_sb, in_=mneg.rearrange("(s p) k -> p s k", p=128))
            nc.scalar.dma_start(out=i128_sb, in_=i128[:, :])

            def cvap(name, i):
                return cv[:, CV[name] + i:CV[name] + i + 1]

            # ---------------- layernorm (x source: dram f32 or sbuf bf16) ------
            def ln(src_dram, src_sb, col0, ncols, stat_off, out_tile, gname, btname,
                   after_tile=None, sq_on_act=False):
                """src [C, *]; normalizes cols [col0, col0+ncols) writing fp8
                (affine g/bt) into out_tile[:, cc, same cols]. Stats broadcast
                via stats_d rows at stat_off+col."""
                nt = ncols // 512
                for t in range(nt):
                    cs = slice(col0 + 512 * t, col0 + 512 * (t + 1))
                    stat_ps = pss.tile([33, 512], F32, tag="sps")
                    mu_ps = stat_ps[0:1, :]
                    sq_ps = stat_ps[32:33, :]
                    xbt = xb_p.tile([128, 8, 512], BF16, tag="xb")
                    for cc in range(8):
                        if src_dram is not None:
                            xt = xin_p.tile([128, 512], F32, tag="x")
                            nc.sync.dma_start(
                                out=xt, in_=src_dram[128 * cc:128 * (cc + 1), cs])
                            nc.vector.tensor_copy(out=xbt[:, cc, :], in_=xt)
                        else:
                            nc.vector.tensor_copy(
                                out=xbt[:, cc, :], in_=src_sb[:, cc, cs])
                        sq = xin_p.tile([128, 512], BF16, tag="sq")
                        if sq_on_act:
                            nc.scalar.activation(out=sq, in_=xbt[:, cc, :],
                                                 func=AF.Square)
                        else:
                            nc.vector.tensor_mul(sq, xbt[:, cc, :], xbt[:, cc, :])
                        nc.tensor.matmul(mu_ps, ones1, xbt[:, cc, :],
                                         start=(cc == 0), stop=(cc == 7))
                        nc.tensor.matmul(sq_ps, ones1, sq,
                                         start=(cc == 0), stop=(cc == 7))
                    mu = rows_p.tile([1, 512], F32, tag="mu")
                    ex2 = rows_p.tile([1, 512], F32, tag="t2")
                    nc.vector.tensor_scalar_mul(mu, mu_ps, 1.0 / C)
                    nc.vector.tensor_scalar_mul(ex2, sq_ps, 1.0 / C)
                    var = rows_p.tile([1, 512], F32, tag="t3")
                    nc.vector.scalar_tensor_tensor(
                        out=var, in0=mu, scalar=-1.0, in1=mu, op0=ALU.mult, op1=ALU.mult)
                    nc.vector.tensor_add(var, var, ex2)
                    std = rows_p.tile([1, 512], F32, tag="t2")
                    nc.scalar.activation(out=std, in_=var, func=AF.Sqrt, bias=eps1)
                    arow = rows_p.tile([1, 512], BF16, tag="ab16")
                    nc.vector.reciprocal(out=arow, in_=std)
                    brow = rows_p.tile([1, 512], BF16, tag="ab16b")
                    nc.vector.scalar_tensor_tensor(
                        out=brow, in0=mu, scalar=-1.0, in1=arow,
                        op0=ALU.mult, op1=ALU.mult)
                    so = stat_off + col0 + 512 * t
                    nc.sync.dma_start(out=stats_d[0, so:so + 512][None, :], in_=arow)
                    nc.sync.dma_start(out=stats_d[1, so:so + 512][None, :], in_=brow)
                    a_b = bc_p.tile([128, 512], BF16, tag="a_b")
                    b_b = bc_p.tile([128, 512], BF16, tag="b_b")
                    nc.sync.dma_start(out=a_b, in_=bass.AP(
                        tensor=stats_d[0][None, :].tensor,
                        offset=so, ap=[[0, 128], [1, 512]]))
                    nc.sync.dma_start(out=b_b, in_=bass.AP(
                        tensor=stats_d[1][None, :].tensor,
                        offset=(T + 1024) + so, ap=[[0, 128], [1, 512]]))
                    for cc in range(8):
                        t1 = xin_p.tile([128, 512], BF16, tag="t1")
                        nc.vector.tensor_mul(t1, xbt[:, cc, :], a_b)
                        u = xin_p.tile([128, 512], BF16, tag="u")
                        nc.vector.tensor_add(u, t1, b_b)
                        nc.gpsimd.tensor_scalar(
                            out_tile[:, cc, cs], u, cvap(gname, cc), cvap(btname, cc),
                            op0=ALU.mult, op1=ALU.add)
                    if after_tile is not None:
                        after_tile(t)

            def emit_v(m):
                for fh in range(4):
                    vps = ps1.tile([128, 256], F32, tag="pmid")
                    for g in range(4):
                        nc.tensor.matmul(
                            vps, hT[:, 2 * g:2 * g + 2, 128 * m:128 * (m + 1)],
                            wv_sb[:, fh, g, :, :],
                            start=(g == 0), stop=(g == 3), perf_mode=DRM,
                            skip_group_check=True)
                    nc.vector.scalar_tensor_tensor(
                        out=v_ext[:, m, 4 * fh:4 * fh + 4, 0:HD],
                        in0=vps.rearrange("p (h d) -> p h d", h=4), scalar=WSI,
                        in1=bvb_sb[:, 256 * fh:256 * (fh + 1)].rearrange(
                            "p (h d) -> p h d", h=4),
                        op0=ALU.mult, op1=ALU.add)
                nc.vector.memset(v_ext[:, m, :, HD:HD + 1], 1.0)

            def emit_qh(qh):
                for p in range(8):
                    qtmp = ktmp_p.tile([128, 4, 128], FP8, tag="qt")
                    qps = ps1.tile([128, 4, 128], F32, tag="pmid")
                    for qi4 in range(4):
                        qi = 4 * qh + qi4
                        qc0 = 128 * (2 * qi + 1)
                        for g in range(4):
                            nc.tensor.matmul(
                                qps[:, qi4, :], wq_sb[:, p, g, :, :],
                                hT[:, 2 * g:2 * g + 2, qc0:qc0 + 128],
                                start=(g == 0), stop=(g == 3), perf_mode=DRM,
                                skip_group_check=True)
                    nc.vector.tensor_scalar(
                        qtmp.rearrange("p a b -> p (a b)"),
                        qps.rearrange("p a b -> p (a b)"),
                        WSI, cvap("bq", p), op0=ALU.mult, op1=ALU.add)
                    for odd in range(2):
                        h = 2 * p + odd
                        t4, h4 = h // 4, h % 4
                        for i in range(2):
                            nc.gpsimd.dma_start(
                                out=qst[t4][32 * h4:32 * h4 + 32, i,
                                            4 * qh:4 * qh + 4, :],
                                in_=qtmp[64 * odd + 32 * i:64 * odd + 32 * i + 32,
                                         :, :])

            def emit_k_half(th):
                for p in range(8):
                    ktmp = ktmp_p.tile([128, 1024], FP8, tag="kt")
                    for tt in range(2):
                        t = 2 * th + tt
                        kps = ps1.tile([128, 512], F32, tag="pmid")
                        for half in range(2):
                            cs = slice(512 * t + 256 * half,
                                       512 * t + 256 * (half + 1))
                            for g in range(4):
                                nc.tensor.matmul(
                                    kps[:, 256 * half:256 * (half + 1)],
                                    wk_sb[:, p, g, :, :], hT[:, 2 * g:2 * g + 2, cs],
                                    start=(g == 0), stop=(g == 3), perf_mode=DRM,
                                    skip_group_check=True)
                        nc.vector.tensor_scalar_mul(
                            ktmp[:, 512 * tt:512 * (tt + 1)], kps, WSI)
                    for odd in range(2):
                        h = 2 * p + odd
                        t4, h4 = h // 4, h % 4
                        for i in range(2):
                            nc.gpsimd.dma_start(
                                out=kst[t4][32 * h4:32 * h4 + 32, i,
                                            1024 * th:1024 * (th + 1)],
                                in_=ktmp[64 * odd + 32 * i:64 * odd + 32 * i + 32, :])

            def emit_k_quarter(t):
                for p in range(8):
                    ktq = ktmp_p.tile([128, 512], FP8, tag="kt")
                    kps = ps1.tile([128, 512], F32, tag="pmid")
                    for half in range(2):
                        cs = slice(512 * t + 256 * half, 512 * t + 256 * (half + 1))
                        for g in range(4):
                            nc.tensor.matmul(
                                kps[:, 256 * half:256 * (half + 1)],
                                wk_sb[:, p, g, :, :], hT[:, 2 * g:2 * g + 2, cs],
                                start=(g == 0), stop=(g == 3), perf_mode=DRM,
                                skip_group_check=True)
                    nc.vector.tensor_scalar_mul(ktq, kps, WSI)
                    for odd in range(2):
                        h = 2 * p + odd
                        t4, h4 = h // 4, h % 4
                        for i in range(2):
                            nc.sync.dma_start(
                                out=kst[t4][32 * h4:32 * h4 + 32, i,
                                            512 * t:512 * (t + 1)],
                                in_=ktq[64 * odd + 32 * i:64 * odd + 32 * i + 32, :])

            def emit_qp(qi0):
                for p in range(8):
                    qtmp = ktmp_p.tile([128, 2, 128], FP8, tag="qt")
                    qps = ps1.tile([128, 2, 128], F32, tag="pmid")
                    for qi4 in range(2):
                        qi = qi0 + qi4
                        qc0 = 128 * (2 * qi + 1)
                        for g in range(4):
                            nc.tensor.matmul(
                                qps[:, qi4, :], wq_sb[:, p, g, :, :],
                                hT[:, 2 * g:2 * g + 2, qc0:qc0 + 128],
                                start=(g == 0), stop=(g == 3), perf_mode=DRM,
                                skip_group_check=True)
                    nc.vector.tensor_scalar(
                        qtmp.rearrange("p a b -> p (a b)"),
                        qps.rearrange("p a b -> p (a b)"),
                        WSI, cvap("bq", p), op0=ALU.mult, op1=ALU.add)
                    for odd in range(2):
                        h = 2 * p + odd
                        t4, h4 = h // 4, h % 4
                        for i in range(2):
                            nc.sync.dma_start(
                                out=qst[t4][32 * h4:32 * h4 + 32, i,
                                            qi0:qi0 + 2, :],
                                in_=qtmp[64 * odd + 32 * i:64 * odd + 32 * i + 32,
                                         :, :])

            def ln1_after(t):
                for m in range(4 * t, 4 * t + 4):
                    emit_v(m)
                if t == 1:
                    emit_k_half(0)
                    emit_qh(0)
                if t == 3:
                    emit_k_half(1)
                    emit_qh(1)

            ln(xT, None, 0, T, 0, hT, "g1", "bt1", after_tile=ln1_after, sq_on_act=True)

            if debug:
                for cc in range(8):
                    for t in range(4):
                        dt_ = out_p.tile([128, 512], F32, tag="dbg")
                        nc.vector.tensor_copy(out=dt_, in_=hT[:, cc, 512 * t:512 * (t + 1)])
                        nc.sync.dma_start(
                            out=dbg["hT"][128 * cc:128 * (cc + 1), 512 * t:512 * (t + 1)],
                            in_=dt_)


            if debug:
                for m in range(16):
                    dt_ = out_p.tile([128, H * HD], F32, tag="dbgv")
                    nc.vector.tensor_copy(
                        out=dt_.rearrange("p (h d) -> p h d", h=H),
                        in_=v_ext[:, m, :, 0:HD])
                    nc.sync.dma_start(out=dbg["v"][128 * m:128 * (m + 1), :], in_=dt_)

            # ---------------- attention + proj + LN2 + MLP, pipelined by qi ----
            def attn_qi(qi):
                L = 2 * qi + 2
                for hhalf in range(2):
                    yall = psy.tile([65, 8, 128], F32, tag="yall")
                    for hh in range(8):
                        h = 8 * hhalf + hh
                        t4, h4 = h // 4, h % 4
                        se = se_p.tile([128, 16, 128], FP8, tag="se")
                        for off in range(0, L, 8):
                            pl = min(8, L - off)
                            sps = pss.tile([128, 8, 128], F32, tag="sps")
                            for mi in range(pl):
                                m = off + mi
                                masked = qi >= 1 and m >= 2 * qi
                                nc.tensor.matmul(
                                    sps[:, mi, :],
                                    kst[t4][32 * h4:32 * h4 + 32, :,
                                            128 * m:128 * (m + 1)],
                                    qst[t4][32 * h4:32 * h4 + 32, :, qi, :],
                                    start=True, stop=not masked, perf_mode=DRM,
                                    tile_position=(32 * h4, 0),
                                    skip_group_check=True)
                                if masked:
                                    slot = 2 * (qi - 1) + (m - 2 * qi)
                                    nc.tensor.matmul(
                                        sps[:, mi, :], mneg_sb[:, slot, :], i128_sb,
                                        start=False, stop=True,
                                        skip_group_check=True)
                            nc.scalar.activation(
                                out=se[:, off:off + pl, :], in_=sps[:, 0:pl, :],
                                func=AF.Exp, scale=float(SCALE))
                        for g in range(L // 2):
                            nc.tensor.matmul(
                                yall[0:65, hh, :],
                                v_ext[:, 2 * g:2 * g + 2, h, 0:65],
                                se[:, 2 * g:2 * g + 2, :],
                                start=(g == 0 and hh % 4 == 0),
                                stop=(g == L // 2 - 1 and hh % 4 == 3),
                                perf_mode=DRM, skip_group_check=True)
                    # normalize 8 heads at once; free yall (PSUM) early by
                    # staging sums+values to SBUF, broadcast recips via PE
                    rs = sml_p.tile([1, 1024], BF16, tag="rs")
                    nc.vector.reciprocal(
                        out=rs.rearrange("p (o a b) -> p o a b", o=2, a=4),
                        in_=yall[64:65, :, :].rearrange("p (a o) b -> p o a b",
                                                        o=2))
                    ysb = sml_p.tile([64, 8, 128], BF16, tag="ysb")
                    nc.vector.tensor_copy(out=ysb, in_=yall[0:64, :, :])
                    yv = ysb.rearrange("p (a o) b -> p a o b", o=2)
                    for o in range(2):
                        rbps = ps1.tile([64, 512], F32, tag="pmid")
                        nc.tensor.matmul(rbps, ones64, rs[:, 512 * o:512 * (o + 1)],
                                         start=True, stop=True)
                        rv = rbps.rearrange("p (a b) -> p a b", a=4)
                        if o == 0:
                            nc.vector.tensor_mul(
                                yT[0:64, 4 * hhalf:4 * hhalf + 4,
                                   128 * qi:128 * (qi + 1)],
                                yv[:, :, 0, :], rv)
                        else:
                            yo = sml_p.tile([64, 4, 128], FP8, tag="yo")
                            nc.vector.tensor_mul(yo, yv[:, :, 1, :], rv)
                            nc.sync.dma_start(
                                out=yT[64:128, 4 * hhalf:4 * hhalf + 4,
                                       128 * qi:128 * (qi + 1)],
                                in_=yo)

            def proj_t(t):
                for co in range(8):
                    pps = ps1.tile([128, 512], F32, tag="pmid")
                    for half in range(2):
                        qs = slice(512 * t + 256 * half, 512 * t + 256 * (half + 1))
                        for g in range(4):
                            nc.tensor.matmul(
                                pps[:, 256 * half:256 * (half + 1)],
                                wp_sb[:, co, g, :, :], yT[:, 2 * g:2 * g + 2, qs],
                                start=(g == 0), stop=(g == 3), perf_mode=DRM,
                                skip_group_check=True)
                    xq = xin_p.tile([128, 512], F32, tag="xq")
                    nc.sync.dma_start(out=xq, in_=bass.AP(
                        tensor=xT[:, :].tensor,
                        offset=(128 * co) * T + 128 * (8 * t + 1),
                        ap=[[T, 128], [256, 4], [1, 128]]))
                    s_ = xin_p.tile([128, 512], F32, tag="s_")
                    nc.vector.scalar_tensor_tensor(
                        out=s_, in0=pps, scalar=WSI, in1=xq,
                        op0=ALU.mult, op1=ALU.add)
                    nc.gpsimd.tensor_scalar_add(
                        x2[:, co, 512 * t:512 * (t + 1)], s_, cvap("bp", co))
                    if debug:
                        dt_ = out_p.tile([128, 512], F32, tag="dbg")
                        nc.vector.tensor_scalar_add(dt_, s_, cvap("bp", co))
                        nc.sync.dma_start(
                            out=dbg["x2T"][128 * co:128 * (co + 1),
                                           512 * t:512 * (t + 1)],
                            in_=dt_)

            def fc1_t(t):
                gA = big.tile([128, 32, 512], FP8, tag="A")  # reuses hT slot
                for f in range(32):
                    w1_sl = w1s_p.tile([128, 4, 2, 128], FP8, tag="w1")
                    nc.sync.dma_start(out=w1_sl, in_=w1[:, f, :, :, :])
                    fps = ps1.tile([128, 512], F32, tag="pmid")
                    for half in range(2):
                        qs = slice(512 * t + 256 * half, 512 * t + 256 * (half + 1))
                        for g in range(4):
                            nc.tensor.matmul(
                                fps[:, 256 * half:256 * (half + 1)],
                                w1_sl[:, g, :, :], h2T[:, 2 * g:2 * g + 2, qs],
                                start=(g == 0), stop=(g == 3), perf_mode=DRM,
                                skip_group_check=True)
                    nc.scalar.activation(
                        out=gA[:, f, :], in_=fps, func=AF.Gelu,
                        bias=cvap("b1", f), scale=WSI)
                return gA

            def fc2_t(t, gA):
                for co in range(8):
                    ops_ = ps1.tile([128, 512], F32, tag="pmid")
                    w2_sl = w2s_p.tile([128, 16, 2, 128], FP8, tag="w2")
                    nc.sync.dma_start(out=w2_sl, in_=w2[:, co, :, :, :])
                    for half in range(2):
                        for g in range(16):
                            nc.tensor.matmul(
                                ops_[:, 256 * half:256 * (half + 1)],
                                w2_sl[:, g, :, :],
                                gA[:, 2 * g:2 * g + 2,
                                   256 * half:256 * (half + 1)],
                                start=(g == 0), stop=(g == 15), perf_mode=DRM,
                                skip_group_check=True)
                    o1 = xin_p.tile([128, 512], F32, tag="s_")
                    nc.vector.scalar_tensor_tensor(
                        out=o1, in0=ops_, scalar=WSI,
                        in1=x2[:, co, 512 * t:512 * (t + 1)],
                        op0=ALU.mult, op1=ALU.add)
                    o2 = out_p.tile([128, 512], F32, tag="o2")
                    nc.gpsimd.tensor_scalar_add(o2, o1, cvap("b2", co))
                    nc.sync.dma_start(
                        out=outT[128 * co:128 * (co + 1), 512 * t:512 * (t + 1)],
                        in_=o2)

            for qi in range(4):
                attn_qi(qi)
            attn_qi(4)
            proj_t(0)
            attn_qi(5)
            ln(None, x2, 0, 512, T, h2T, "g2", "bt2")
            attn_qi(6)
            gA0 = fc1_t(0)
            attn_qi(7)
            proj_t(1)
            ln(None, x2, 512, 512, T, h2T, "g2", "bt2")
            fc2_t(0, gA0)

            if debug:
                for p in range(8):
                    for t in range(2):
                        dt_ = out_p.tile([128, 512], F32, tag="dbg")
                        nc.vector.tensor_copy(
                            out=dt_, in_=yT[:, p, 512 * t:512 * (t + 1)])
                        nc.sync.dma_start(
                            out=dbg["yT"][128 * p:128 * (p + 1),
                                          512 * t:512 * (t + 1)],
                            in_=dt_)

            gA1 = fc1_t(1)
            fc2_t(1, gA1)

    return nc


# ===================== host side =====================

def make_mneg(rho, cond):
    """14 slots [(qi,ms) for qi in 1..7 for ms in 0,1], each stored transposed
    [q, k] fp8: 0 where allowed, -240 where masked. Key chunk at permuted
    position m corresponds to natural chunk (pairswap(m) if rho==0 else m)."""
    out = np.zeros((14, 128, 128), dtype=np.float32)
    for qi in range(1, 8):
        qnat = 2 * qi + rho          # natural q chunk
        qrows = 128 * qnat + np.arange(128)
        for ms in range(2):
            m = 2 * qi + ms          # permuted key-chunk position
            n = (m ^ 1) if rho == 0 else m   # natural key chunk
            krows = 128 * n + np.arange(128)
            i = qrows[None, :]       # [1, q] -> transposed layout [q, k] idx [k, q]?
            # allowed(i=query, j=key); stored slot[q, k]
            qq = qrows[:, None]
            kk = krows[None, :]
            allowed = (kk < cond) | ((qq >= cond) & (kk >= cond) & (kk <= qq))
            out[2 * (qi - 1) + ms] = np.where(allowed, 0.0, MNEG)
    return out.reshape(14 * 128, 128).astype(E4)


def pack_dr_w(W, gdim, fblk):
    """W [1024 or 4096, F] -> [128, nf, g, 2, fblk] with c = 256g+128i+p."""
    Cin, F = W.shape
    ng = Cin // 256
    nf = F // fblk
    out = np.empty((128, nf, ng, 2, fblk), dtype=E4)
    Ws = (W * WS).astype(E4)
    for fi in range(nf):
        blk = Ws[:, fblk * fi:fblk * (fi + 1)]          # [Cin, fblk]
        r = blk.reshape(ng, 2, 128, fblk)                # g, i, p, f
        out[:, fi] = r.transpose(2, 0, 1, 3)             # p, g, i, f
    return np.ascontiguousarray(out)


def perm_cols(rho):
    """column permutation of T at 128 granularity: rho=0 pairswap, rho=1 id."""
    chunks = np.arange(16)
    if rho == 0:
        chunks = chunks ^ 1
    idx = (128 * chunks[:, None] + np.arange(128)[None, :]).reshape(-1)
    return idx


def shard_inputs(inputs):
    x = np.asarray(inputs["x"], np.float32)
    cond = int(np.asarray(inputs["cond_len"]))
    cvec = np.zeros((128, 88), np.float32)
    for name, key in [("bq", "bq"), ("bp", "bp"), ("b2", "b2"), ("g1", "ln1_g"),
                      ("bt1", "ln1_b"), ("g2", "ln2_g"), ("bt2", "ln2_b")]:
        v = np.asarray(inputs[key], np.float32)
        cvec[:, CV[name]:CV[name] + 8] = v.reshape(8, 128).T
    cvec[:, CV["b1"]:CV["b1"] + 32] = np.asarray(
        inputs["b1"], np.float32).reshape(32, 128).T
    common = {
        "wq": pack_dr_w(np.asarray(inputs["Wq"], np.float32), 4, 128),
        "wk": pack_dr_w(np.asarray(inputs["Wk"], np.float32), 4, 128),
        "wv": pack_dr_w(np.asarray(inputs["Wv"], np.float32), 4, 256),
        "wp": pack_dr_w(np.asarray(inputs["Wp"], np.float32), 4, 128),
        "w1": pack_dr_w(np.asarray(inputs["W1"], np.float32), 4, 128),
        "w2": pack_dr_w(np.asarray(inputs["W2"], np.float32), 16, 128),
        "cvec": cvec,
        "bvb": np.ascontiguousarray(np.broadcast_to(
            np.asarray(inputs["bv"], np.float32), (128, C))).astype(
                ml_dtypes.bfloat16),
        "i128": np.eye(128, dtype=np.float32).astype(E4),
    }
    in_maps, row_sets = [], []
    for c in range(8):
        b, rho = c // 2, c % 2
        m = dict(common)
        cols = perm_cols(rho)
        m["xT"] = np.ascontiguousarray(x[b].T[:, cols])
        m["mneg"] = make_mneg(rho, cond)
        rows = np.concatenate(
            [np.arange(128 * (2 * qi + rho), 128 * (2 * qi + rho) + 128)
             for qi in range(8)])
        row_sets.append((b, rows))
        in_maps.append(m)
    return in_maps, row_sets


_cached_nc = {}


def get_nc(debug=False):
    if debug not in _cached_nc:
        _cached_nc[debug] = build_nc(debug=debug)
    return _cached_nc[debug]


def run(inputs, debug=False):
    nc = get_nc(debug=debug)
    in_maps, row_sets = shard_inputs(inputs)
    res = run_bass_kernel_spmd(nc, in_maps, core_ids=list(range(8)))
    x = np.asarray(inputs["x"], np.float32)
    out = np.empty_like(x)
    for c in range(8):
        b, rows = row_sets[c]
        out[b][rows] = res.results[c]["outT"].T
    return out, res, row_sets


def kernel(**inputs):
    out, _, _ = run(inputs, debug=False)
    return out
